# revision 1
# baseline (speedup 1.0000x reference)
"""Trainium2 Bass kernel for MixformerAttention (sparse attention).

Problem shape (hardcoded from the problem spec):
  x [B=64, N=320, C=768], W_qkv [768, 2304], W_proj [768, 768], b_proj [768]
  H=12 heads, Dh=64, template length L = t_h*t_w = 64, search = 256.

Sharding: data-parallel over batch across 8 NeuronCores (8 batches/core).

Per-core pipeline (batches processed in pairs of 2 -> 640 tokens = 5x128):
  1. DMA x pair -> SBUF, PE-transpose to x^T (C on partitions), cast fp16
  2. QKV^T matmul for q,k  ->  q^T,k^T [Dh, tok] per head (fp16); odd heads
     are DMA-shifted to partition base 0 (matmul operands must not live at
     SBUF partition base 64 - implicit PE row-tiling crashes the exec unit)
     V natural matmul      ->  v [tok, head, 65] with a ones column (fp16)
  3. Per batch: S^T = k q^T per head/key-chunk -> exp (ACT, scale=1/8) -> fp16
     PV: O[q, h-slot] = expS^T.T @ [v|1]  (ones column yields softmax denom);
     query chunks (template 64 | search 128 | search 128), all at psum base 0
     normalize rows by 1/denominator -> attn [tok, C] fp16
  4. attn^T via DMA tile-transposes, proj matmul + bias -> out fp32 -> DMA out

All matmuls use fp16 operands (1 cycle/row on the PE, fp32 PSUM accumulate).
"""

import functools

import numpy as np

import concourse.bacc as bacc
import concourse.mybir as mybir
from concourse.bass_utils import run_bass_kernel_spmd
from concourse.masks import make_identity
from concourse.tile import TileContext

F32 = mybir.dt.float32
F16 = mybir.dt.float16

NCORES = 8
B, N, C = 64, 320, 768
H, DH = 12, 64
KS = C // 128  # 6 contraction subtiles
B_CORE = B // NCORES  # 8 batches per core
PAIR_TOK = 2 * N  # 640
NPAIR = B_CORE // 2  # 4
TOK_CORE = B_CORE * N  # 2560
SLOT = 85  # psum column stride per head in PV output (6 heads/bank)

# key chunks of one batch's 320 tokens (partition base 0 each)
KT_CHUNKS = [(0, 128), (128, 128), (256, 64)]
# query chunks: template | search 0:128 | search 128:256
Q_CHUNKS = [(0, 64), (64, 128), (192, 128)]
# token chunks used for the projection / output rows
P_CHUNKS = [(0, 128), (128, 128), (256, 64)]


def _emit_pair(nc, tc, pools, consts, x_ap, out_ap, p):
    """Emit instructions for one pair of batches (640 tokens)."""
    psum = pools["psum"]
    psum_b = pools["psum_b"]
    wqkv16, wproj16, bias_bc, ident32, ident16 = consts

    # ---- load x pair and build x^T (fp16, C on partitions) ----
    x_nat = pools["x_nat"].tile([128, 5, C], F32, tag="x_nat")
    nc.sync.dma_start(
        x_nat[:],
        x_ap[p * PAIR_TOK : (p + 1) * PAIR_TOK, :].rearrange(
            "(t p) f -> p t f", p=128
        ),
    )
    xT = pools["xT"].tile([128, KS, PAIR_TOK], F16, tag="xT")
    for fc in range(KS):
        ps_a = psum.tile([128, 512], F32, tag="ps")
        ps_b = psum.tile([128, 512], F32, tag="ps")
        for t in range(5):
            dst = ps_a[:, t * 128 : (t + 1) * 128] if t < 4 else ps_b[:, 0:128]
            nc.tensor.transpose(
                dst, x_nat[:, t, fc * 128 : (fc + 1) * 128], ident32
            )
        nc.vector.tensor_copy(xT[:, fc, 0:512], ps_a[:, 0:512])
        nc.vector.tensor_copy(xT[:, fc, 512:640], ps_b[:, 0:128])

    # ---- q^T / k^T : out^T = W_qk.T @ x^T, features on partitions ----
    # feature chunk fc covers q (fc 0..5) then k (fc 6..11); head pair per chunk
    qkT = pools["qkT"].tile([128, 2 * KS, PAIR_TOK], F16, tag="qkT")
    qkTo = pools["qkTo"].tile([64, 2 * KS, PAIR_TOK], F16, tag="qkTo")
    for fc in range(2 * KS):
        ps1 = psum.tile([128, 512], F32, tag="ps")
        ps2 = psum.tile([128, 512], F32, tag="ps")
        for ks in range(KS):
            lhsT = wqkv16[:, ks, fc * 128 : (fc + 1) * 128]
            nc.tensor.matmul(
                ps1[:, 0:512],
                lhsT=lhsT,
                rhs=xT[:, ks, 0:512],
                start=(ks == 0),
                stop=(ks == KS - 1),
            )
            nc.tensor.matmul(
                ps2[:, 0:128],
                lhsT=lhsT,
                rhs=xT[:, ks, 512:640],
                start=(ks == 0),
                stop=(ks == KS - 1),
            )
        nc.scalar.copy(qkT[:, fc, 0:512], ps1[:, 0:512])
        nc.scalar.copy(qkT[:, fc, 512:640], ps2[:, 0:128])
        # odd head of this chunk lives at partitions 64..128; shift it
        # to partition base 0 (matmul operands must not sit at base 64)
        nc.sync.dma_start(qkTo[0:64, fc, :], qkT[64:128, fc, :])

    # ---- per batch within the pair ----
    for b2 in range(2):
        bb = p * 2 + b2  # batch index within this core
        btok = b2 * N

        # ---- v natural [tok, h, 0:65] with ones column ----
        va = pools["v"].tile([128, 3, H, 66], F16, tag="v")
        for ci, (off, sz) in enumerate(KT_CHUNKS):
            for half, (n0, nw) in enumerate([(0, 512), (512, 256)]):
                ps = psum.tile([128, 512], F32, tag="ps")
                for ks in range(KS):
                    nc.tensor.matmul(
                        ps[:sz, 0:nw],
                        lhsT=xT[:, ks, btok + off : btok + off + sz],
                        rhs=wqkv16[:, ks, 2 * C + n0 : 2 * C + n0 + nw],
                        start=(ks == 0),
                        stop=(ks == KS - 1),
                    )
                h0, nh = (0, 8) if half == 0 else (8, 4)
                nc.vector.tensor_copy(
                    va[:sz, ci, h0 : h0 + nh, 0:64],
                    ps[:sz, 0:nw].rearrange("p (h d) -> p h d", d=64),
                )
        nc.vector.memset(va[:, :, :, 64], 1.0)

        def kT(h):
            src = qkT if h % 2 == 0 else qkTo
            return src[0:64, KS + h // 2, :]

        def qT(h):
            src = qkT if h % 2 == 0 else qkTo
            return src[0:64, h // 2, :]

        # ---- search scores S^T [kt, q] + exp (4 heads per 2-bank psum) ----
        es_ci = []
        for ci, (off, sz) in enumerate(KT_CHUNKS):
            es = pools["expS"].tile([128, H, 256], F16, tag="expS")
            es_ci.append(es)
            for hg in range(3):
                ps = psum_b.tile([128, 1024], F32, tag="psb")
                for j in range(4):
                    h = 4 * hg + j
                    nc.tensor.matmul(
                        ps[:sz, j * 256 : (j + 1) * 256],
                        lhsT=kT(h)[:, btok + off : btok + off + sz],
                        rhs=qT(h)[:, btok + 64 : btok + 320],
                        start=True,
                        stop=True,
                    )
                nc.scalar.activation(
                    es[:sz, 4 * hg : 4 * hg + 4, :],
                    ps[:sz, 0:1024].rearrange("p (h q) -> p h q", q=256),
                    mybir.ActivationFunctionType.Exp,
                    scale=0.125,
                )

        # ---- template scores (template attends only to template keys) ----
        esm = pools["expSmt"].tile([64, H, 64], F16, tag="expSmt")
        for hg in range(2):
            ps = psum.tile([128, 512], F32, tag="ps")
            for j in range(6):
                h = 6 * hg + j
                nc.tensor.matmul(
                    ps[0:64, j * 64 : (j + 1) * 64],
                    lhsT=kT(h)[:, btok : btok + 64],
                    rhs=qT(h)[:, btok : btok + 64],
                    start=True,
                    stop=True,
                )
            nc.scalar.activation(
                esm[:, 6 * hg : 6 * hg + 6, :],
                ps[0:64, 0:384].rearrange("p (h q) -> p h q", q=64),
                mybir.ActivationFunctionType.Exp,
                scale=0.125,
            )

        # ---- PV + softmax normalization -> attn rows (fp16) ----
        # attn chunk qg holds rows [Q_CHUNKS[qg]] of the batch at base 0
        attn = pools["attn"].tile([128, 3, C], F16, tag="attn")
        nc.vector.memset(attn[64:128, 0, :], 0.0)  # pad rows read by DMA-T

        def normalize(po, qsz, qg, half):
            po_v = po[:qsz, 0:510].rearrange("p (h s) -> p h s", s=SLOT)
            rcp = pools["rcp"].tile([128, 8], F32, tag="rcp")
            nc.vector.reciprocal(rcp[:qsz, 0:6], po_v[:, :, 64])
            nc.vector.tensor_tensor(
                attn[:qsz, qg, half * 384 : (half + 1) * 384].rearrange(
                    "p (h d) -> p h d", d=64
                ),
                po_v[:, :, 0:64],
                rcp[:qsz, 0:6, None].to_broadcast([qsz, 6, 64]),
                mybir.AluOpType.mult,
            )

        for half in range(2):
            # template rows (batch rows 0..64)
            po = psum.tile([128, 512], F32, tag="ps")
            for j in range(6):
                h = 6 * half + j
                nc.tensor.matmul(
                    po[0:64, j * SLOT : j * SLOT + 65],
                    lhsT=esm[:, h, 0:64],
                    rhs=va[0:64, 0, h, 0:65],
                    start=True,
                    stop=True,
                )
            normalize(po, 64, 0, half)
            # search rows: q chunks of 128
            for qg in (1, 2):
                qlo = Q_CHUNKS[qg][0] - 64
                po = psum.tile([128, 512], F32, tag="ps")
                for j in range(6):
                    h = 6 * half + j
                    for ci, (koff, ksz) in enumerate(KT_CHUNKS):
                        nc.tensor.matmul(
                            po[0:128, j * SLOT : j * SLOT + 65],
                            lhsT=es_ci[ci][:ksz, h, qlo : qlo + 128],
                            rhs=va[:ksz, ci, h, 0:65],
                            start=(ci == 0),
                            stop=(ci == 2),
                        )
                normalize(po, 128, qg, half)

        # ---- attn^T via PE transposes (fp16) ----
        attnT = pools["attnT"].tile([128, KS, N], F16, tag="attnT")
        for fc in range(KS):
            pt = pools["psum_h"].tile([128, 512], F16, tag="psh")
            for qg, dst0 in ((0, 0), (1, 64), (2, 192)):
                qsz = Q_CHUNKS[qg][1]
                nc.tensor.transpose(
                    pt[:, dst0 : dst0 + qsz],
                    attn[0:qsz, qg, fc * 128 : (fc + 1) * 128],
                    ident16[:qsz, :qsz],
                )
            nc.vector.tensor_copy(attnT[:, fc, 0:N], pt[:, 0:N])

        # ---- output projection + bias ----
        out_sb = pools["out"].tile([128, 3, C], F32, tag="out")
        for qc, (qoff, qsz) in enumerate(P_CHUNKS):
            for half, (n0, nw) in enumerate([(0, 512), (512, 256)]):
                pp = psum.tile([128, 512], F32, tag="ps")
                for ks in range(KS):
                    nc.tensor.matmul(
                        pp[:qsz, 0:nw],
                        lhsT=attnT[:, ks, qoff : qoff + qsz],
                        rhs=wproj16[:, ks, n0 : n0 + nw],
                        start=(ks == 0),
                        stop=(ks == KS - 1),
                    )
                nc.vector.tensor_tensor(
                    out_sb[:qsz, qc, n0 : n0 + nw],
                    pp[:qsz, 0:nw],
                    bias_bc[:qsz, n0 : n0 + nw],
                    mybir.AluOpType.add,
                )
        row0 = bb * N
        nc.sync.dma_start(
            out_ap[row0 : row0 + 256, :].rearrange("(t p) f -> p t f", p=128),
            out_sb[:, 0:2, :],
        )
        nc.sync.dma_start(out_ap[row0 + 256 : row0 + N, :], out_sb[0:64, 2, :])


def build_kernel():
    nc = bacc.Bacc("TRN2", target_bir_lowering=False)
    x_t = nc.dram_tensor("x", [TOK_CORE, C], F32, kind="ExternalInput")
    wqkv_t = nc.dram_tensor("W_qkv", [C, 3 * C], F32, kind="ExternalInput")
    wproj_t = nc.dram_tensor("W_proj", [C, C], F32, kind="ExternalInput")
    bias_t = nc.dram_tensor("b_proj", [C], F32, kind="ExternalInput")
    out_t = nc.dram_tensor("out", [TOK_CORE, C], F32, kind="ExternalOutput")

    with TileContext(nc) as tc:
        import contextlib

        with contextlib.ExitStack() as ctx:
            pools = {
                "const": ctx.enter_context(tc.tile_pool(name="const", bufs=1)),
                "stage": ctx.enter_context(tc.tile_pool(name="stage", bufs=2)),
                "x_nat": ctx.enter_context(tc.tile_pool(name="x_nat", bufs=1)),
                "xT": ctx.enter_context(tc.tile_pool(name="xT", bufs=2)),
                "qkT": ctx.enter_context(tc.tile_pool(name="qkT", bufs=2)),
                "qkTo": ctx.enter_context(tc.tile_pool(name="qkTo", bufs=2)),
                "v": ctx.enter_context(tc.tile_pool(name="v", bufs=2)),
                "expS": ctx.enter_context(tc.tile_pool(name="expS", bufs=3)),
                "expSmt": ctx.enter_context(tc.tile_pool(name="expSmt", bufs=2)),
                "attn": ctx.enter_context(tc.tile_pool(name="attn", bufs=2)),
                "attnT": ctx.enter_context(tc.tile_pool(name="attnT", bufs=2)),
                "out": ctx.enter_context(tc.tile_pool(name="out", bufs=1)),
                "rcp": ctx.enter_context(tc.tile_pool(name="rcp", bufs=4)),
                "psum": ctx.enter_context(
                    tc.tile_pool(name="psum", bufs=3, space="PSUM")
                ),
                "psum_h": ctx.enter_context(
                    tc.tile_pool(name="psum_h", bufs=1, space="PSUM")
                ),
                "psum_b": ctx.enter_context(
                    tc.tile_pool(name="psum_b", bufs=2, space="PSUM")
                ),
            }
            const = pools["const"]

            # constants: fp16 weights, broadcast bias, identity
            wqkv16 = const.tile([128, KS, 3 * C], F16, tag="wqkv16")
            wproj16 = const.tile([128, KS, C], F16, tag="wproj16")
            bias_bc = const.tile([128, C], F32, tag="bias_bc")
            ident32 = const.tile([128, 128], F32, tag="ident32")
            ident16 = const.tile([128, 128], F16, tag="ident16")
            make_identity(nc, ident32)
            make_identity(nc, ident16)

            for ks in range(KS):
                st = pools["stage"].tile([128, 3 * C], F32, tag="stage")
                nc.sync.dma_start(st[:], wqkv_t.ap()[ks * 128 : (ks + 1) * 128, :])
                nc.vector.tensor_copy(wqkv16[:, ks, :], st[:])
            for ks in range(KS):
                st = pools["stage"].tile([128, 3 * C], F32, tag="stage")
                nc.sync.dma_start(
                    st[:, 0:C], wproj_t.ap()[ks * 128 : (ks + 1) * 128, :]
                )
                nc.vector.tensor_copy(wproj16[:, ks, :], st[:, 0:C])
            brow = pools["stage"].tile([128, 3 * C], F32, tag="stage")
            nc.sync.dma_start(brow[0:1, 0:C], bias_t.ap().unsqueeze(0))
            nc.gpsimd.partition_broadcast(bias_bc[:, :], brow[0:1, 0:C])

            consts = (wqkv16, wproj16, bias_bc, ident32, ident16)
            for p in range(NPAIR):
                _emit_pair(nc, tc, pools, consts, x_t.ap(), out_t.ap(), p)

    nc.compile()
    return nc


@functools.cache
def _get_nc():
    return build_kernel()


def kernel(**inputs):
    x = np.ascontiguousarray(np.asarray(inputs["x"], dtype=np.float32))
    wqkv = np.ascontiguousarray(np.asarray(inputs["W_qkv"], dtype=np.float32))
    wproj = np.ascontiguousarray(np.asarray(inputs["W_proj"], dtype=np.float32))
    bias = np.ascontiguousarray(np.asarray(inputs["b_proj"], dtype=np.float32))
    t_h = int(inputs.get("t_h", 8))
    t_w = int(inputs.get("t_w", 8))
    assert t_h * t_w == 64, "kernel built for template length 64"
    assert x.shape == (B, N, C)

    nc = _get_nc()
    in_maps = [
        {
            "x": x[c * B_CORE : (c + 1) * B_CORE].reshape(TOK_CORE, C),
            "W_qkv": wqkv,
            "W_proj": wproj,
            "b_proj": bias,
        }
        for c in range(NCORES)
    ]
    res = run_bass_kernel_spmd(nc, in_maps, core_ids=list(range(NCORES)))
    out = np.concatenate(
        [r["out"].reshape(B_CORE, N, C) for r in res.results], axis=0
    )
    return out.astype(np.float32)


if __name__ == "__main__":
    _get_nc()
    print("kernel built OK")



# revision 2
# speedup vs baseline: 1.0233x; 1.0233x over previous
"""Trainium2 Bass kernel for MixformerAttention (sparse attention) — v2.

Problem shape (hardcoded):
  x [B=64, N=320, C=768], W_qkv [768, 2304], W_proj [768, 768], b_proj [768]
  H=12 heads, Dh=64, template L=64, search=256. DP over batch on 8 cores.

v2 redesign vs baseline (empirically driven by the NTFF trace):
  * Every matmul whose lhsT had only 64 partition rows (scores, template,
    PV tail) paid a ~100ns serialized LDWEIGHTS. All attention operands are
    now zero-padded to full 128 contraction rows:
      - qTp: per-head q tiles [128, tok], data in the head's native 64-row
        half, zeros in the other half (DMA'd from the qk psum drain).
      - kT stays packed 2-heads/chunk; the junk half multiplies the zero
        half of qTp, contributing 0.
      - es/esm/va key-padded the same way (persistent tiles, pads zeroed
        once at startup).
  * attn^T computed with regular matmuls (lhsT=attn chunk, rhs=identity):
    weight-load overlaps, unlike transpose-mode where the data IS the
    weight load. Template/search chunks overlap-packed into one psum strip
    so each fc drains with a single copy.
  * Software-pipelined emission: scores/exp of batch g interleave with
    PV/attnT/proj of batch g-1 (deferred thunks), so the Act-engine exp
    latency never stalls the PE.
  * PSUM budget exactly 8 banks: pool_g 3x[128,768] + pool_s 1x[128,1024].
"""

import contextlib
import functools

import numpy as np

import concourse.bacc as bacc
import concourse.mybir as mybir
from concourse.bass_utils import run_bass_kernel_spmd
from concourse.masks import make_identity
from concourse.tile import TileContext

F32 = mybir.dt.float32
F16 = mybir.dt.float16

NCORES = 8
B, N, C = 64, 320, 768
H, DH = 12, 64
KS = C // 128  # 6 contraction subtiles
B_CORE = B // NCORES  # 8
PAIR_TOK = 2 * N  # 640
NPAIR = B_CORE // 2  # 4
TOK_CORE = B_CORE * N  # 2560
SLOT = 85  # psum col stride per head in PV output (6 heads in 510 cols)

KT_CHUNKS = [(0, 128), (128, 128), (256, 64)]  # key chunks per batch
P_CHUNKS = [(0, 128), (128, 128), (256, 64)]  # proj token chunks per batch


def build_kernel():
    nc = bacc.Bacc("TRN2", target_bir_lowering=False)
    x_t = nc.dram_tensor("x", [TOK_CORE, C], F32, kind="ExternalInput")
    wqkv_t = nc.dram_tensor("W_qkv", [C, 3 * C], F32, kind="ExternalInput")
    wproj_t = nc.dram_tensor("W_proj", [C, C], F32, kind="ExternalInput")
    bias_t = nc.dram_tensor("b_proj", [C], F32, kind="ExternalInput")
    out_t = nc.dram_tensor("out", [TOK_CORE, C], F32, kind="ExternalOutput")
    x_ap, out_ap = x_t.ap(), out_t.ap()

    with TileContext(nc) as tc:
        with contextlib.ExitStack() as ctx:
            P = {
                "const": ctx.enter_context(tc.tile_pool(name="const", bufs=1)),
                "stage": ctx.enter_context(tc.tile_pool(name="stage", bufs=2)),
                "x_nat": ctx.enter_context(tc.tile_pool(name="x_nat", bufs=5)),
                "xT": ctx.enter_context(tc.tile_pool(name="xT", bufs=2)),
                "qkfc": ctx.enter_context(tc.tile_pool(name="qkfc", bufs=3)),
                "outst": ctx.enter_context(tc.tile_pool(name="outst", bufs=2)),
                "rcp": ctx.enter_context(tc.tile_pool(name="rcp", bufs=4)),
                "pg": ctx.enter_context(tc.tile_pool(name="pg", bufs=3, space="PSUM")),
                "ps": ctx.enter_context(tc.tile_pool(name="ps", bufs=1, space="PSUM")),
            }
            const = P["const"]

            # ---- persistent constants ----
            wqkv16 = const.tile([128, KS, 3 * C], F16, tag="wqkv16")
            wproj16 = const.tile([128, KS, C], F16, tag="wproj16")
            bias_bc = const.tile([128, C], F32, tag="bias_bc")
            ident32 = const.tile([128, 128], F32, tag="ident32")
            ident16 = const.tile([128, 128], F16, tag="ident16")
            make_identity(nc, ident32)
            make_identity(nc, ident16)

            # ---- persistent double-slotted activation tiles ----
            # per-head padded q (slot = pair parity)
            qTp = const.tile([128, 2, H, PAIR_TOK], F16, tag="qTp")
            # packed kT feature chunks (2 heads per chunk)
            kTpk = const.tile([128, 2, KS, PAIR_TOK], F16, tag="kTpk")
            # v natural with ones column (slot = batch parity)
            va = const.tile([128, 2, 3, H, 66], F16, tag="va")
            # exp(scores) for search queries [key, h, q]  (256 = search q)
            es = const.tile([128, 2, 3, H, 256], F16, tag="es")
            # exp(scores) template [key<=64 padded, h, q0:64]
            esm = const.tile([128, 2, H, 64], F16, tag="esm")
            # attention rows (template 64 padded | search 128 | search 128)
            attn = const.tile([128, 2, 3, C], F16, tag="attn")
            # attn^T per batch [C-part, tok]
            attnT = const.tile([128, 2, KS, N], F16, tag="attnT")

            # ---- one-time pad zeroing (split engines, overlaps weight DMA) ----
            nc.gpsimd.memset(qTp[64:128, :, 0:H:2, :], 0.0)  # even heads pad
            nc.vector.memset(qTp[0:64, :, 1:H:2, :], 0.0)  # odd heads pad
            nc.vector.memset(es[64:128, :, 2, :, :], 0.0)  # key chunk 2 pad
            nc.vector.memset(esm[64:128, :, :, :], 0.0)  # template key pad
            nc.vector.memset(attn[64:128, :, 0, :], 0.0)  # template row pad
            nc.gpsimd.memset(va[64:128, :, 2, :, :], 0.0)  # v key chunk 2 pad
            nc.vector.memset(va[:, :, :, :, 64], 1.0)  # ones column
            nc.vector.memset(va[:, :, :, :, 65], 0.0)  # stride pad

            # ---- weights: DMA + cast (split across engines for startup) ----
            for ks in range(KS):
                st = P["stage"].tile([128, 3 * C], F32, tag="stage")
                nc.sync.dma_start(st[:], wqkv_t.ap()[ks * 128 : (ks + 1) * 128, :])
                eng = (nc.scalar.copy, nc.vector.tensor_copy, nc.gpsimd.tensor_copy)[
                    ks % 3
                ]
                eng(wqkv16[:, ks, :], st[:])
            for ks in range(KS):
                st = P["stage"].tile([128, 3 * C], F32, tag="stage")
                nc.sync.dma_start(st[:, 0:C], wproj_t.ap()[ks * 128 : (ks + 1) * 128, :])
                eng = (nc.scalar.copy, nc.vector.tensor_copy, nc.gpsimd.tensor_copy)[
                    ks % 3
                ]
                eng(wproj16[:, ks, :], st[:, 0:C])
            brow = P["stage"].tile([128, 3 * C], F32, tag="stage")
            nc.sync.dma_start(brow[0:1, 0:C], bias_t.ap().unsqueeze(0))
            nc.gpsimd.partition_broadcast(bias_bc[:, :], brow[0:1, 0:C])

            # ================= emission helpers =================

            def emit_xdma(p):
                """DMA the 5 x chunks of pair p into x_nat pool tiles."""
                tiles = []
                for t in range(5):
                    xt = P["x_nat"].tile([128, C], F32, tag="xn")
                    r0 = p * PAIR_TOK + t * 128
                    nc.sync.dma_start(xt[:], x_ap[r0 : r0 + 128, :])
                    tiles.append(xt)
                return tiles

            def emit_A(p, xtiles):
                """Transpose x pair p -> xT (fp32 PE transposes)."""
                sl = p % 2
                xT = P["xT"].tile([128, KS, PAIR_TOK], F16, tag="xT")
                pieces = []
                for fc in range(KS):
                    def piece(fc=fc, xT=xT, xtiles=xtiles):
                        tg = P["pg"].tile([128, 1024], F32, tag="pg")
                        for t in range(5):
                            nc.tensor.transpose(
                                tg[:, t * 128 : (t + 1) * 128],
                                xtiles[t][:, fc * 128 : (fc + 1) * 128],
                                ident32,
                            )
                        nc.vector.tensor_copy(xT[:, fc, :], tg[:, 0:640])
                    pieces.append(piece)
                return xT, pieces

            def emit_B(p, xT):
                """qk matmuls for pair p -> qTp (DMA) and kTpk (drain)."""
                sl = p % 2
                for fc in range(2 * KS):
                    tg = P["pg"].tile([128, 1024], F32, tag="pg")
                    for ks in range(KS):
                        nc.tensor.matmul(
                            tg[:, 0:512],
                            lhsT=wqkv16[:, ks, fc * 128 : (fc + 1) * 128],
                            rhs=xT[:, ks, 0:512],
                            start=(ks == 0),
                            stop=(ks == KS - 1),
                        )
                    for ks in range(KS):
                        nc.tensor.matmul(
                            tg[:, 512:640],
                            lhsT=wqkv16[:, ks, fc * 128 : (fc + 1) * 128],
                            rhs=xT[:, ks, 512:640],
                            start=(ks == 0),
                            stop=(ks == KS - 1),
                        )
                    if fc < KS:  # q features -> padded per-head tiles via DMA
                        qf = P["qkfc"].tile([128, PAIR_TOK], F16, tag="qkfc")
                        nc.vector.tensor_copy(qf[:], tg[:, 0:640])
                        nc.sync.dma_start(qTp[0:64, sl, 2 * fc, :], qf[0:64, :])
                        nc.sync.dma_start(qTp[64:128, sl, 2 * fc + 1, :], qf[64:128, :])
                    else:  # k features -> packed tile directly (alternate engines)
                        if fc % 2 == 0:
                            nc.vector.tensor_copy(kTpk[:, sl, fc - KS, :], tg[:, 0:640])
                        else:
                            nc.scalar.copy(kTpk[:, sl, fc - KS, :], tg[:, 0:640])

            def make_C(g):
                """v matmuls for batch g (3 psum tiles)."""
                sl, b2 = g % 2, g % 2
                btok = (g % 2) * N
                psl = (g // 2) % 2
                pieces = []
                for ci, (off, sz) in enumerate(KT_CHUNKS):
                    def piece(ci=ci, off=off, sz=sz):
                        xT = xT_cur[g // 2]
                        tg = P["pg"].tile([128, 1024], F32, tag="pg")
                        for ks in range(KS):
                            nc.tensor.matmul(
                                tg[:sz, 0:512],
                                lhsT=xT[:, ks, btok + off : btok + off + sz],
                                rhs=wqkv16[:, ks, 2 * C : 2 * C + 512],
                                start=(ks == 0),
                                stop=(ks == KS - 1),
                            )
                        for ks in range(KS):
                            nc.tensor.matmul(
                                tg[:sz, 512:768],
                                lhsT=xT[:, ks, btok + off : btok + off + sz],
                                rhs=wqkv16[:, ks, 2 * C + 512 : 3 * C],
                                start=(ks == 0),
                                stop=(ks == KS - 1),
                            )
                        nc.scalar.copy(
                            va[:sz, sl, ci, :, 0:64],
                            tg[:sz, 0:768].rearrange("p (h d) -> p h d", d=64),
                        )
                    pieces.append(piece)
                return pieces

            def emit_D(g, fillers):
                """Scores + exp for batch g, interleaving filler pieces."""
                sl = g % 2
                psl = (g // 2) % 2
                btok = (g % 2) * N
                fi = 0
                for ci, (koff, ksz) in enumerate(KT_CHUNKS):
                    for hg in range(3):
                        # two heads per matmul: both heads' padded q side by
                        # side (N=512); the packed kT chunk's parity halves
                        # each hit their own head, zeros kill cross terms
                        psc = P["ps"].tile([128, 2, 2, 256], F32, tag="ps")
                        for hp in range(2):
                            h0 = hg * 4 + hp * 2
                            nc.tensor.matmul(
                                psc[:ksz, hp, :, :],
                                lhsT=kTpk[:, psl, h0 // 2, btok + koff : btok + koff + ksz],
                                rhs=qTp[:, psl, h0 : h0 + 2, btok + 64 : btok + 320],
                                start=True,
                                stop=True,
                            )
                        nc.scalar.activation(
                            es[:ksz, sl, ci, hg * 4 : hg * 4 + 4, :],
                            psc[:ksz, :, :, :].rearrange("p a b q -> p (a b) q"),
                            mybir.ActivationFunctionType.Exp,
                            scale=0.125,
                        )
                        for _ in range(2):
                            if fi < len(fillers):
                                fillers[fi]()
                                fi += 1
                while fi < len(fillers):
                    fillers[fi]()
                    fi += 1

            def make_E(g):
                """Template scores + exp for batch g (2 pieces)."""
                sl = g % 2
                psl = (g // 2) % 2
                btok = (g % 2) * N
                holder = {}

                def mm_piece():
                    tg = P["pg"].tile([128, 1024], F32, tag="pg")
                    holder["tg"] = tg
                    for hp in range(6):
                        h0 = 2 * hp
                        nc.tensor.matmul(
                            tg[0:64, h0 * 64 : (h0 + 2) * 64],
                            lhsT=kTpk[:, psl, hp, btok : btok + 64],
                            rhs=qTp[:, psl, h0 : h0 + 2, btok : btok + 64],
                            start=True,
                            stop=True,
                        )

                def exp_piece():
                    tg = holder["tg"]
                    nc.scalar.activation(
                        esm[0:64, sl, :, :],
                        tg[0:64, 0:768].rearrange("p (h q) -> p h q", q=64),
                        mybir.ActivationFunctionType.Exp,
                        scale=0.125,
                    )

                return [mm_piece, exp_piece]

            def _normalize(tg, qsz, qg, half, sl):
                po_v = tg[:qsz, 0:510].rearrange("p (h s) -> p h s", s=SLOT)
                rcp = P["rcp"].tile([128, 8], F32, tag="rcp")
                nc.vector.reciprocal(rcp[:qsz, 0:6], po_v[:, :, 64])
                nc.vector.tensor_tensor(
                    attn[:qsz, sl, qg, half * 384 : (half + 1) * 384].rearrange(
                        "p (h d) -> p h d", d=64
                    ),
                    po_v[:, :, 0:64],
                    rcp[:qsz, 0:6, None].to_broadcast([qsz, 6, 64]),
                    mybir.AluOpType.mult,
                )

            def make_FGH(g):
                """PV + normalize + attn^T + proj for batch g (deferred)."""
                sl = g % 2
                pieces = []

                # template PV (2 pieces, one per head-half)
                for half in range(2):
                    def tpv(half=half):
                        tg = P["pg"].tile([128, 1024], F32, tag="pg")
                        for j in range(6):
                            h = half * 6 + j
                            nc.tensor.matmul(
                                tg[0:64, j * SLOT : j * SLOT + 65],
                                lhsT=esm[:, sl, h, 0:64],
                                rhs=va[:, sl, 0, h, 0:65],
                                start=True,
                                stop=True,
                            )
                        _normalize(tg, 64, 0, half, sl)
                    pieces.append(tpv)

                # search PV (4 pieces: qg x half)
                for qg in (1, 2):
                    for half in range(2):
                        def spv(qg=qg, half=half):
                            tg = P["pg"].tile([128, 1024], F32, tag="pg")
                            for j in range(6):
                                h = half * 6 + j
                                for ci in range(3):
                                    nc.tensor.matmul(
                                        tg[0:128, j * SLOT : j * SLOT + 65],
                                        lhsT=es[:, sl, ci, h, (qg - 1) * 128 : qg * 128],
                                        rhs=va[:, sl, ci, h, 0:65],
                                        start=(ci == 0),
                                        stop=(ci == 2),
                                    )
                            _normalize(tg, 128, qg, half, sl)
                        pieces.append(spv)

                # attn^T via regular matmuls (6 pieces)
                for fc in range(KS):
                    def at(fc=fc):
                        tg = P["pg"].tile([128, 1024], F32, tag="pg")
                        # overlap-packed: qg0 -> 0:128 (real 0:64), qg1 -> 64:192,
                        # qg2 -> 192:320
                        for qg, dst0 in ((0, 0), (1, 64), (2, 192)):
                            nc.tensor.matmul(
                                tg[:, dst0 : dst0 + 128],
                                lhsT=attn[0:128, sl, qg, fc * 128 : (fc + 1) * 128],
                                rhs=ident16[:, 0:128],
                                start=True,
                                stop=True,
                            )
                        nc.vector.tensor_copy(attnT[:, sl, fc, 0:N], tg[:, 0:N])
                    pieces.append(at)

                # proj + bias + out DMA (3 pieces)
                row0 = g * N
                for qc, (qoff, qsz) in enumerate(P_CHUNKS):
                    def pj(qc=qc, qoff=qoff, qsz=qsz):
                        tg = P["pg"].tile([128, 1024], F32, tag="pg")
                        for ks in range(KS):
                            nc.tensor.matmul(
                                tg[:qsz, 0:512],
                                lhsT=attnT[:, sl, ks, qoff : qoff + qsz],
                                rhs=wproj16[:, ks, 0:512],
                                start=(ks == 0),
                                stop=(ks == KS - 1),
                            )
                        for ks in range(KS):
                            nc.tensor.matmul(
                                tg[:qsz, 512:768],
                                lhsT=attnT[:, sl, ks, qoff : qoff + qsz],
                                rhs=wproj16[:, ks, 512:768],
                                start=(ks == 0),
                                stop=(ks == KS - 1),
                            )
                        ost = P["outst"].tile([128, C], F32, tag="outst")
                        nc.vector.tensor_tensor(
                            ost[:qsz, :], tg[:qsz, 0:768], bias_bc[:qsz, :],
                            mybir.AluOpType.add,
                        )
                        nc.sync.dma_start(
                            out_ap[row0 + qoff : row0 + qoff + qsz, :], ost[:qsz, :]
                        )
                    pieces.append(pj)
                return pieces

            # ================= main schedule =================
            xT_cur = {}
            xtiles = emit_xdma(0)
            xT0, a_pieces = emit_A(0, xtiles)
            xT_cur[0] = xT0
            for pc in a_pieces:
                pc()

            stash = []
            for p in range(NPAIR):
                for pc in stash:  # F/G/H of batch 2p-1
                    pc()
                stash = []
                if p + 1 < NPAIR:
                    xtiles = emit_xdma(p + 1)
                emit_B(p, xT_cur[p])
                g0, g1 = 2 * p, 2 * p + 1
                for pc in make_C(g0):
                    pc()
                fill0 = make_C(g1) + make_E(g0)
                if p + 1 < NPAIR:
                    xT1, a_next = emit_A(p + 1, xtiles)
                    xT_cur[p + 1] = xT1
                    fill0 = fill0 + a_next
                emit_D(g0, fill0)
                stash0 = make_FGH(g0)
                emit_D(g1, stash0 + make_E(g1))
                stash = make_FGH(g1)
            for pc in stash:
                pc()

    nc.compile()
    return nc


@functools.cache
def _get_nc():
    return build_kernel()


def kernel(**inputs):
    x = np.ascontiguousarray(np.asarray(inputs["x"], dtype=np.float32))
    wqkv = np.ascontiguousarray(np.asarray(inputs["W_qkv"], dtype=np.float32))
    wproj = np.ascontiguousarray(np.asarray(inputs["W_proj"], dtype=np.float32))
    bias = np.ascontiguousarray(np.asarray(inputs["b_proj"], dtype=np.float32))
    t_h = int(inputs.get("t_h", 8))
    t_w = int(inputs.get("t_w", 8))
    assert t_h * t_w == 64, "kernel built for template length 64"
    assert x.shape == (B, N, C)

    nc = _get_nc()
    in_maps = [
        {
            "x": x[c * B_CORE : (c + 1) * B_CORE].reshape(TOK_CORE, C),
            "W_qkv": wqkv,
            "W_proj": wproj,
            "b_proj": bias,
        }
        for c in range(NCORES)
    ]
    res = run_bass_kernel_spmd(nc, in_maps, core_ids=list(range(NCORES)))
    out = np.concatenate(
        [r["out"].reshape(B_CORE, N, C) for r in res.results], axis=0
    )
    return out.astype(np.float32)


if __name__ == "__main__":
    _get_nc()
    print("kernel_v2 built OK")


# revision 3
# speedup vs baseline: 1.1116x; 1.0863x over previous
"""Trainium2 Bass kernel for MixformerAttention (sparse attention) — v2.

Problem shape (hardcoded):
  x [B=64, N=320, C=768], W_qkv [768, 2304], W_proj [768, 768], b_proj [768]
  H=12 heads, Dh=64, template L=64, search=256. DP over batch on 8 cores.

v2 redesign vs baseline (empirically driven by the NTFF trace):
  * Every matmul whose lhsT had only 64 partition rows (scores, template,
    PV tail) paid a ~100ns serialized LDWEIGHTS. All attention operands are
    now zero-padded to full 128 contraction rows:
      - qTp: per-head q tiles [128, tok], data in the head's native 64-row
        half, zeros in the other half (DMA'd from the qk psum drain).
      - kT stays packed 2-heads/chunk; the junk half multiplies the zero
        half of qTp, contributing 0.
      - es/esm/va key-padded the same way (persistent tiles, pads zeroed
        once at startup).
  * attn^T computed with regular matmuls (lhsT=attn chunk, rhs=identity):
    weight-load overlaps, unlike transpose-mode where the data IS the
    weight load. Template/search chunks overlap-packed into one psum strip
    so each fc drains with a single copy.
  * Software-pipelined emission: scores/exp of batch g interleave with
    PV/attnT/proj of batch g-1 (deferred thunks), so the Act-engine exp
    latency never stalls the PE.
  * PSUM budget exactly 8 banks: pool_g 3x[128,768] + pool_s 1x[128,1024].
"""

import contextlib
import functools

import numpy as np

import concourse.bacc as bacc
import concourse.mybir as mybir
from concourse.bass_utils import run_bass_kernel_spmd
from concourse.masks import make_identity
from concourse.tile import TileContext

F32 = mybir.dt.float32
F16 = mybir.dt.float16

NCORES = 8
B, N, C = 64, 320, 768
H, DH = 12, 64
KS = C // 128  # 6 contraction subtiles
B_CORE = B // NCORES  # 8
PAIR_TOK = 2 * N  # 640
NPAIR = B_CORE // 2  # 4
TOK_CORE = B_CORE * N  # 2560
SLOT = 85  # psum col stride per head in PV output (6 heads in 510 cols)

KT_CHUNKS = [(0, 128), (128, 128), (256, 64)]  # key chunks per batch
P_CHUNKS = [(0, 128), (128, 128), (256, 64)]  # proj token chunks per batch


def build_kernel():
    nc = bacc.Bacc("TRN2", target_bir_lowering=False)
    x_t = nc.dram_tensor("x", [TOK_CORE, C], F32, kind="ExternalInput")
    wqkv_t = nc.dram_tensor("W_qkv", [C, 3 * C], F32, kind="ExternalInput")
    wproj_t = nc.dram_tensor("W_proj", [C, C], F32, kind="ExternalInput")
    bias_t = nc.dram_tensor("b_proj", [C], F32, kind="ExternalInput")
    out_t = nc.dram_tensor("out", [TOK_CORE, C], F32, kind="ExternalOutput")
    x_ap, out_ap = x_t.ap(), out_t.ap()

    with TileContext(nc) as tc:
        with contextlib.ExitStack() as ctx:
            P = {
                "const": ctx.enter_context(tc.tile_pool(name="const", bufs=1)),
                "stage": ctx.enter_context(tc.tile_pool(name="stage", bufs=3)),
                "stagep": ctx.enter_context(tc.tile_pool(name="stagep", bufs=1)),
                "x_nat": ctx.enter_context(tc.tile_pool(name="x_nat", bufs=5)),
                "xT": ctx.enter_context(tc.tile_pool(name="xT", bufs=2)),
                "qkfc": ctx.enter_context(tc.tile_pool(name="qkfc", bufs=3)),
                "outst": ctx.enter_context(tc.tile_pool(name="outst", bufs=2)),
                "rcp": ctx.enter_context(tc.tile_pool(name="rcp", bufs=4)),
                "pg": ctx.enter_context(tc.tile_pool(name="pg", bufs=3, space="PSUM")),
                "ps": ctx.enter_context(tc.tile_pool(name="ps", bufs=1, space="PSUM")),
            }
            const = P["const"]

            # ---- persistent constants ----
            wqkv16 = const.tile([128, KS, 3 * C], F16, tag="wqkv16")
            wproj16 = const.tile([128, KS, C], F16, tag="wproj16")
            bias_bc = const.tile([128, C], F32, tag="bias_bc")
            ident32 = const.tile([128, 128], F32, tag="ident32")
            ident16 = const.tile([128, 128], F16, tag="ident16")
            make_identity(nc, ident32)
            make_identity(nc, ident16)

            # ---- persistent double-slotted activation tiles ----
            # per-head padded q (slot = pair parity)
            qTp = const.tile([128, 2, H, PAIR_TOK], F16, tag="qTp")
            # packed kT feature chunks (2 heads per chunk)
            kTpk = const.tile([128, 2, KS, PAIR_TOK], F16, tag="kTpk")
            # v natural with ones column (slot = batch parity)
            va = const.tile([128, 2, 3, H, 66], F16, tag="va")
            # exp(scores) for search queries [key, h, q]  (256 = search q)
            es = const.tile([128, 2, 3, H, 256], F16, tag="es")
            # exp(scores) template [key<=64 padded, h, q0:64]
            esm = const.tile([128, 2, H, 64], F16, tag="esm")
            # attention rows (template 64 padded | search 128 | search 128)
            attn = const.tile([128, 2, 3, C], F16, tag="attn")
            # attn^T per batch [C-part, tok]
            attnT = const.tile([128, 2, KS, N], F16, tag="attnT")

            def emit_weight_load():
                # wqkv in 12 half-chunks: parallel DMAs, casts off the DVE
                for ks in range(KS):
                    for hh in range(2):
                        st = P["stage"].tile([128, 3 * C // 2], F32, tag="stage")
                        nc.sync.dma_start(
                            st[:],
                            wqkv_t.ap()[
                                ks * 128 : (ks + 1) * 128,
                                hh * (3 * C // 2) : (hh + 1) * (3 * C // 2),
                            ],
                        )
                        eng = (nc.scalar.copy, nc.gpsimd.tensor_copy)[(2 * ks + hh) % 2]
                        eng(
                            wqkv16[:, ks, hh * (3 * C // 2) : (hh + 1) * (3 * C // 2)],
                            st[:],
                        )
                for ks in range(KS):
                    st = P["stagep"].tile([128, C], F32, tag="stagep")
                    nc.sync.dma_start(st[:], wproj_t.ap()[ks * 128 : (ks + 1) * 128, :])
                    eng = (nc.scalar.copy, nc.gpsimd.tensor_copy)[ks % 2]
                    eng(wproj16[:, ks, :], st[:])
                brow = P["stagep"].tile([128, C], F32, tag="stagep")
                nc.sync.dma_start(brow[0:1, 0:C], bias_t.ap().unsqueeze(0))
                nc.gpsimd.partition_broadcast(bias_bc[:, :], brow[0:1, 0:C])

            def emit_pads():
                # one-time pad zeroing (emitted after A(0) so the DVE queue
                # drains xT first; WAW deps keep correctness)
                nc.gpsimd.memset(qTp[64:128, :, 0:H:2, :], 0.0)  # even heads
                nc.vector.memset(qTp[0:64, :, 1:H:2, :], 0.0)  # odd heads
                nc.vector.memset(es[64:128, :, 2, :, :], 0.0)  # key chunk 2
                nc.vector.memset(esm[64:128, :, :, :], 0.0)  # template keys
                nc.vector.memset(attn[64:128, :, 0, :], 0.0)  # template rows
                nc.gpsimd.memset(va[64:128, :, 2, :, :], 0.0)  # v key chunk 2
                nc.vector.memset(va[:, :, :, :, 64], 1.0)  # ones column
                nc.vector.memset(va[:, :, :, :, 65], 0.0)  # stride pad

            # ================= emission helpers =================

            def emit_xdma(p):
                """DMA the 5 x chunks of pair p into x_nat pool tiles."""
                tiles = []
                for t in range(5):
                    xt = P["x_nat"].tile([128, C], F32, tag="xn")
                    r0 = p * PAIR_TOK + t * 128
                    nc.sync.dma_start(xt[:], x_ap[r0 : r0 + 128, :])
                    tiles.append(xt)
                return tiles

            def emit_A(p, xtiles):
                """Transpose x pair p -> xT (fp32 PE transposes)."""
                sl = p % 2
                xT = P["xT"].tile([128, KS, PAIR_TOK], F16, tag="xT")
                pieces = []
                for fc in range(KS):
                    def piece(fc=fc, xT=xT, xtiles=xtiles):
                        tg = P["pg"].tile([128, 1024], F32, tag="pg")
                        for t in range(5):
                            nc.tensor.transpose(
                                tg[:, t * 128 : (t + 1) * 128],
                                xtiles[t][:, fc * 128 : (fc + 1) * 128],
                                ident32,
                            )
                        nc.vector.tensor_copy(xT[:, fc, :], tg[:, 0:640])
                    pieces.append(piece)
                return xT, pieces

            def emit_B(p, xT):
                """qk matmuls for pair p -> qTp (DMA) and kTpk (drain)."""
                sl = p % 2
                for fc in range(2 * KS):
                    tg = P["pg"].tile([128, 1024], F32, tag="pg")
                    for ks in range(KS):
                        nc.tensor.matmul(
                            tg[:, 0:512],
                            lhsT=wqkv16[:, ks, fc * 128 : (fc + 1) * 128],
                            rhs=xT[:, ks, 0:512],
                            start=(ks == 0),
                            stop=(ks == KS - 1),
                        )
                    for ks in range(KS):
                        nc.tensor.matmul(
                            tg[:, 512:640],
                            lhsT=wqkv16[:, ks, fc * 128 : (fc + 1) * 128],
                            rhs=xT[:, ks, 512:640],
                            start=(ks == 0),
                            stop=(ks == KS - 1),
                        )
                    if fc < KS:  # q features -> padded per-head tiles via DMA
                        qf = P["qkfc"].tile([128, PAIR_TOK], F16, tag="qkfc")
                        nc.vector.tensor_copy(qf[:], tg[:, 0:640])
                        nc.sync.dma_start(qTp[0:64, sl, 2 * fc, :], qf[0:64, :])
                        nc.sync.dma_start(qTp[64:128, sl, 2 * fc + 1, :], qf[64:128, :])
                    else:  # k features -> packed tile directly (alternate engines)
                        if fc % 2 == 0:
                            nc.vector.tensor_copy(kTpk[:, sl, fc - KS, :], tg[:, 0:640])
                        else:
                            nc.scalar.copy(kTpk[:, sl, fc - KS, :], tg[:, 0:640])

            def make_C(g):
                """v matmuls for batch g (3 psum tiles)."""
                sl, b2 = g % 2, g % 2
                btok = (g % 2) * N
                psl = (g // 2) % 2
                pieces = []
                for ci, (off, sz) in enumerate(KT_CHUNKS):
                    def piece(ci=ci, off=off, sz=sz):
                        xT = xT_cur[g // 2]
                        tg = P["pg"].tile([128, 1024], F32, tag="pg")
                        for ks in range(KS):
                            nc.tensor.matmul(
                                tg[:sz, 0:512],
                                lhsT=xT[:, ks, btok + off : btok + off + sz],
                                rhs=wqkv16[:, ks, 2 * C : 2 * C + 512],
                                start=(ks == 0),
                                stop=(ks == KS - 1),
                            )
                        for ks in range(KS):
                            nc.tensor.matmul(
                                tg[:sz, 512:768],
                                lhsT=xT[:, ks, btok + off : btok + off + sz],
                                rhs=wqkv16[:, ks, 2 * C + 512 : 3 * C],
                                start=(ks == 0),
                                stop=(ks == KS - 1),
                            )
                        nc.scalar.copy(
                            va[:sz, sl, ci, :, 0:64],
                            tg[:sz, 0:768].rearrange("p (h d) -> p h d", d=64),
                        )
                    pieces.append(piece)
                return pieces

            def emit_D(g, fillers):
                """Scores + exp for batch g, interleaving filler pieces."""
                sl = g % 2
                psl = (g // 2) % 2
                btok = (g % 2) * N
                fi = 0
                for ci, (koff, ksz) in enumerate(KT_CHUNKS):
                    for hg in range(3):
                        # two heads per matmul: both heads' padded q side by
                        # side (N=512); the packed kT chunk's parity halves
                        # each hit their own head, zeros kill cross terms
                        psc = P["ps"].tile([128, 2, 2, 256], F32, tag="ps")
                        for hp in range(2):
                            h0 = hg * 4 + hp * 2
                            nc.tensor.matmul(
                                psc[:ksz, hp, :, :],
                                lhsT=kTpk[:, psl, h0 // 2, btok + koff : btok + koff + ksz],
                                rhs=qTp[:, psl, h0 : h0 + 2, btok + 64 : btok + 320],
                                start=True,
                                stop=True,
                            )
                        nc.scalar.activation(
                            es[:ksz, sl, ci, hg * 4 : hg * 4 + 4, :],
                            psc[:ksz, :, :, :].rearrange("p a b q -> p (a b) q"),
                            mybir.ActivationFunctionType.Exp,
                            scale=0.125,
                        )
                        for _ in range(2):
                            if fi < len(fillers):
                                fillers[fi]()
                                fi += 1
                while fi < len(fillers):
                    fillers[fi]()
                    fi += 1

            def make_E(g):
                """Template scores + exp for batch g (2 pieces)."""
                sl = g % 2
                psl = (g // 2) % 2
                btok = (g % 2) * N
                holder = {}

                def mm_piece():
                    tg = P["pg"].tile([128, 1024], F32, tag="pg")
                    holder["tg"] = tg
                    for hp in range(6):
                        h0 = 2 * hp
                        nc.tensor.matmul(
                            tg[0:64, h0 * 64 : (h0 + 2) * 64],
                            lhsT=kTpk[:, psl, hp, btok : btok + 64],
                            rhs=qTp[:, psl, h0 : h0 + 2, btok : btok + 64],
                            start=True,
                            stop=True,
                        )

                def exp_piece():
                    tg = holder["tg"]
                    nc.scalar.activation(
                        esm[0:64, sl, :, :],
                        tg[0:64, 0:768].rearrange("p (h q) -> p h q", q=64),
                        mybir.ActivationFunctionType.Exp,
                        scale=0.125,
                    )

                return [mm_piece, exp_piece]

            def _normalize(tg, qsz, qg, half, sl):
                po_v = tg[:qsz, 0:510].rearrange("p (h s) -> p h s", s=SLOT)
                rcp = P["rcp"].tile([128, 8], F32, tag="rcp")
                nc.vector.reciprocal(rcp[:qsz, 0:6], po_v[:, :, 64])
                nc.vector.tensor_tensor(
                    attn[:qsz, sl, qg, half * 384 : (half + 1) * 384].rearrange(
                        "p (h d) -> p h d", d=64
                    ),
                    po_v[:, :, 0:64],
                    rcp[:qsz, 0:6, None].to_broadcast([qsz, 6, 64]),
                    mybir.AluOpType.mult,
                )

            def make_FGH(g):
                """PV + normalize + attn^T + proj for batch g (deferred)."""
                sl = g % 2
                pieces = []

                # template PV (2 pieces, one per head-half)
                for half in range(2):
                    def tpv(half=half):
                        tg = P["pg"].tile([128, 1024], F32, tag="pg")
                        for j in range(6):
                            h = half * 6 + j
                            nc.tensor.matmul(
                                tg[0:64, j * SLOT : j * SLOT + 65],
                                lhsT=esm[:, sl, h, 0:64],
                                rhs=va[:, sl, 0, h, 0:65],
                                start=True,
                                stop=True,
                            )
                        _normalize(tg, 64, 0, half, sl)
                    pieces.append(tpv)

                # search PV (4 pieces: qg x half)
                for qg in (1, 2):
                    for half in range(2):
                        def spv(qg=qg, half=half):
                            tg = P["pg"].tile([128, 1024], F32, tag="pg")
                            for j in range(6):
                                h = half * 6 + j
                                for ci in range(3):
                                    nc.tensor.matmul(
                                        tg[0:128, j * SLOT : j * SLOT + 65],
                                        lhsT=es[:, sl, ci, h, (qg - 1) * 128 : qg * 128],
                                        rhs=va[:, sl, ci, h, 0:65],
                                        start=(ci == 0),
                                        stop=(ci == 2),
                                    )
                            _normalize(tg, 128, qg, half, sl)
                        pieces.append(spv)

                # attn^T via regular matmuls (6 pieces)
                for fc in range(KS):
                    def at(fc=fc):
                        tg = P["pg"].tile([128, 1024], F32, tag="pg")
                        # overlap-packed: qg0 -> 0:128 (real 0:64), qg1 -> 64:192,
                        # qg2 -> 192:320
                        for qg, dst0 in ((0, 0), (1, 64), (2, 192)):
                            nc.tensor.matmul(
                                tg[:, dst0 : dst0 + 128],
                                lhsT=attn[0:128, sl, qg, fc * 128 : (fc + 1) * 128],
                                rhs=ident16[:, 0:128],
                                start=True,
                                stop=True,
                            )
                        nc.vector.tensor_copy(attnT[:, sl, fc, 0:N], tg[:, 0:N])
                    pieces.append(at)

                # proj + bias + out DMA (3 pieces)
                row0 = g * N
                for qc, (qoff, qsz) in enumerate(P_CHUNKS):
                    def pj(qc=qc, qoff=qoff, qsz=qsz):
                        tg = P["pg"].tile([128, 1024], F32, tag="pg")
                        for ks in range(KS):
                            nc.tensor.matmul(
                                tg[:qsz, 0:512],
                                lhsT=attnT[:, sl, ks, qoff : qoff + qsz],
                                rhs=wproj16[:, ks, 0:512],
                                start=(ks == 0),
                                stop=(ks == KS - 1),
                            )
                        for ks in range(KS):
                            nc.tensor.matmul(
                                tg[:qsz, 512:768],
                                lhsT=attnT[:, sl, ks, qoff : qoff + qsz],
                                rhs=wproj16[:, ks, 512:768],
                                start=(ks == 0),
                                stop=(ks == KS - 1),
                            )
                        ost = P["outst"].tile([128, C], F32, tag="outst")
                        nc.vector.tensor_tensor(
                            ost[:qsz, :], tg[:qsz, 0:768], bias_bc[:qsz, :],
                            mybir.AluOpType.add,
                        )
                        nc.sync.dma_start(
                            out_ap[row0 + qoff : row0 + qoff + qsz, :], ost[:qsz, :]
                        )
                    pieces.append(pj)
                return pieces

            # ================= main schedule =================
            # x DMAs first (unblock PE transposes ASAP), then the weight
            # block (HBM-bound), pads last so DVE drains xT promptly.
            xT_cur = {}
            xtiles = emit_xdma(0)
            emit_weight_load()
            xT0, a_pieces = emit_A(0, xtiles)
            xT_cur[0] = xT0
            for pc in a_pieces:
                pc()
            emit_pads()

            stash = []
            for p in range(NPAIR):
                for pc in stash:  # F/G/H of batch 2p-1
                    pc()
                stash = []
                if p + 1 < NPAIR:
                    xtiles = emit_xdma(p + 1)
                emit_B(p, xT_cur[p])
                g0, g1 = 2 * p, 2 * p + 1
                for pc in make_C(g0):
                    pc()
                fill0 = make_C(g1) + make_E(g0)
                if p + 1 < NPAIR:
                    xT1, a_next = emit_A(p + 1, xtiles)
                    xT_cur[p + 1] = xT1
                    fill0 = fill0 + a_next
                emit_D(g0, fill0)
                stash0 = make_FGH(g0)
                emit_D(g1, stash0 + make_E(g1))
                stash = make_FGH(g1)
            for pc in stash:
                pc()

    nc.compile()
    return nc


@functools.cache
def _get_nc():
    return build_kernel()


def kernel(**inputs):
    x = np.ascontiguousarray(np.asarray(inputs["x"], dtype=np.float32))
    wqkv = np.ascontiguousarray(np.asarray(inputs["W_qkv"], dtype=np.float32))
    wproj = np.ascontiguousarray(np.asarray(inputs["W_proj"], dtype=np.float32))
    bias = np.ascontiguousarray(np.asarray(inputs["b_proj"], dtype=np.float32))
    t_h = int(inputs.get("t_h", 8))
    t_w = int(inputs.get("t_w", 8))
    assert t_h * t_w == 64, "kernel built for template length 64"
    assert x.shape == (B, N, C)

    nc = _get_nc()
    in_maps = [
        {
            "x": x[c * B_CORE : (c + 1) * B_CORE].reshape(TOK_CORE, C),
            "W_qkv": wqkv,
            "W_proj": wproj,
            "b_proj": bias,
        }
        for c in range(NCORES)
    ]
    res = run_bass_kernel_spmd(nc, in_maps, core_ids=list(range(NCORES)))
    out = np.concatenate(
        [r["out"].reshape(B_CORE, N, C) for r in res.results], axis=0
    )
    return out.astype(np.float32)


if __name__ == "__main__":
    _get_nc()
    print("kernel_v2 built OK")


# revision 4
# speedup vs baseline: 1.1252x; 1.0122x over previous
"""Trainium2 Bass kernel for MixformerAttention (sparse attention) — v2.

Problem shape (hardcoded):
  x [B=64, N=320, C=768], W_qkv [768, 2304], W_proj [768, 768], b_proj [768]
  H=12 heads, Dh=64, template L=64, search=256. DP over batch on 8 cores.

v2 redesign vs baseline (empirically driven by the NTFF trace):
  * Every matmul whose lhsT had only 64 partition rows (scores, template,
    PV tail) paid a ~100ns serialized LDWEIGHTS. All attention operands are
    now zero-padded to full 128 contraction rows:
      - qTp: per-head q tiles [128, tok], data in the head's native 64-row
        half, zeros in the other half (DMA'd from the qk psum drain).
      - kT stays packed 2-heads/chunk; the junk half multiplies the zero
        half of qTp, contributing 0.
      - es/esm/va key-padded the same way (persistent tiles, pads zeroed
        once at startup).
  * attn^T computed with regular matmuls (lhsT=attn chunk, rhs=identity):
    weight-load overlaps, unlike transpose-mode where the data IS the
    weight load. Template/search chunks overlap-packed into one psum strip
    so each fc drains with a single copy.
  * Software-pipelined emission: scores/exp of batch g interleave with
    PV/attnT/proj of batch g-1 (deferred thunks), so the Act-engine exp
    latency never stalls the PE.
  * PSUM budget exactly 8 banks: pool_g 3x[128,768] + pool_s 1x[128,1024].
"""

import contextlib
import functools

import numpy as np

import concourse.bacc as bacc
import concourse.mybir as mybir
from concourse.bass_utils import run_bass_kernel_spmd
from concourse.masks import make_identity
from concourse.tile import TileContext

F32 = mybir.dt.float32
F16 = mybir.dt.float16

NCORES = 8
B, N, C = 64, 320, 768
H, DH = 12, 64
KS = C // 128  # 6 contraction subtiles
B_CORE = B // NCORES  # 8
PAIR_TOK = 2 * N  # 640
NPAIR = B_CORE // 2  # 4
TOK_CORE = B_CORE * N  # 2560
SLOT = 85  # psum col stride per head in PV output (6 heads in 510 cols)

KT_CHUNKS = [(0, 128), (128, 128), (256, 64)]  # key chunks per batch
P_CHUNKS = [(0, 128), (128, 128), (256, 64)]  # proj token chunks per batch


def build_kernel():
    nc = bacc.Bacc("TRN2", target_bir_lowering=False)
    x_t = nc.dram_tensor("x16", [TOK_CORE, C], F16, kind="ExternalInput")
    wqkv_t = nc.dram_tensor("W_qkv16", [C, 3 * C], F16, kind="ExternalInput")
    wproj_t = nc.dram_tensor("W_proj16", [C, C], F16, kind="ExternalInput")
    bias_t = nc.dram_tensor("b_proj", [C], F32, kind="ExternalInput")
    out_t = nc.dram_tensor("out", [TOK_CORE, C], F32, kind="ExternalOutput")
    x_ap, out_ap = x_t.ap(), out_t.ap()

    with TileContext(nc) as tc:
        with contextlib.ExitStack() as ctx:
            P = {
                "const": ctx.enter_context(tc.tile_pool(name="const", bufs=1)),
                "stagep": ctx.enter_context(tc.tile_pool(name="stagep", bufs=1)),
                "x_nat": ctx.enter_context(tc.tile_pool(name="x_nat", bufs=7)),
                "xT": ctx.enter_context(tc.tile_pool(name="xT", bufs=2)),
                "qkfc": ctx.enter_context(tc.tile_pool(name="qkfc", bufs=3)),
                "outst": ctx.enter_context(tc.tile_pool(name="outst", bufs=2)),
                "rcp": ctx.enter_context(tc.tile_pool(name="rcp", bufs=4)),
                "pg": ctx.enter_context(tc.tile_pool(name="pg", bufs=3, space="PSUM")),
                "ps": ctx.enter_context(tc.tile_pool(name="ps", bufs=1, space="PSUM")),
            }
            const = P["const"]

            # ---- persistent constants ----
            wqkv16 = const.tile([128, KS, 3 * C], F16, tag="wqkv16")
            wproj16 = const.tile([128, KS, C], F16, tag="wproj16")
            bias_bc = const.tile([128, C], F32, tag="bias_bc")
            ident32 = const.tile([128, 128], F32, tag="ident32")
            ident16 = const.tile([128, 128], F16, tag="ident16")
            make_identity(nc, ident32)
            make_identity(nc, ident16)

            # ---- persistent double-slotted activation tiles ----
            # per-head padded q (slot = pair parity)
            qTp = const.tile([128, 2, H, PAIR_TOK], F16, tag="qTp")
            # packed kT feature chunks (2 heads per chunk)
            kTpk = const.tile([128, 2, KS, PAIR_TOK], F16, tag="kTpk")
            # v natural with ones column (slot = batch parity)
            va = const.tile([128, 2, 3, H, 66], F16, tag="va")
            # exp(scores) for search queries [key, h, q]  (256 = search q)
            es = const.tile([128, 2, 3, H, 256], F16, tag="es")
            # exp(scores) template [key<=64 padded, h, q0:64]
            esm = const.tile([128, 2, H, 64], F16, tag="esm")
            # attention rows (template 64 padded | search 128 | search 128)
            attn = const.tile([128, 2, 3, C], F16, tag="attn")
            # attn^T per batch [C-part, tok]
            attnT = const.tile([128, 2, KS, N], F16, tag="attnT")

            def emit_weight_load():
                # fp16 weights land directly in their SBUF tiles (host-cast)
                for ks in range(KS):
                    nc.sync.dma_start(
                        wqkv16[:, ks, :], wqkv_t.ap()[ks * 128 : (ks + 1) * 128, :]
                    )
                for ks in range(KS):
                    nc.sync.dma_start(
                        wproj16[:, ks, :], wproj_t.ap()[ks * 128 : (ks + 1) * 128, :]
                    )
                brow = P["stagep"].tile([128, C], F32, tag="stagep")
                nc.sync.dma_start(brow[0:1, 0:C], bias_t.ap().unsqueeze(0))
                nc.gpsimd.partition_broadcast(bias_bc[:, :], brow[0:1, 0:C])

            def emit_pads():
                # one-time pad zeroing (emitted after A(0) so the DVE queue
                # drains xT first; WAW deps keep correctness)
                nc.gpsimd.memset(qTp[64:128, :, 0:H:2, :], 0.0)  # even heads
                nc.vector.memset(qTp[0:64, :, 1:H:2, :], 0.0)  # odd heads
                nc.vector.memset(es[64:128, :, 2, :, :], 0.0)  # key chunk 2
                nc.vector.memset(esm[64:128, :, :, :], 0.0)  # template keys
                nc.vector.memset(attn[64:128, :, 0, :], 0.0)  # template rows
                nc.gpsimd.memset(va[64:128, :, 2, :, :], 0.0)  # v key chunk 2
                nc.vector.memset(va[:, :, :, :, 64], 1.0)  # ones column
                nc.vector.memset(va[:, :, :, :, 65], 0.0)  # stride pad

            # ================= emission helpers =================

            def emit_xdma(p):
                """DMA the 5 x chunks of pair p into x_nat pool tiles."""
                tiles = []
                for t in range(5):
                    xt = P["x_nat"].tile([128, C], F16, tag="xn")
                    r0 = p * PAIR_TOK + t * 128
                    nc.sync.dma_start(xt[:], x_ap[r0 : r0 + 128, :])
                    tiles.append(xt)
                return tiles

            def emit_A(p, xtiles):
                """x -> fp16 (gpsimd) -> x^T via regular matmuls (rhs=ident)."""
                x16s = xtiles  # already fp16 from DRAM
                xT = P["xT"].tile([128, KS, PAIR_TOK], F16, tag="xT")
                pieces = []
                for fc in range(KS):
                    def piece(fc=fc, xT=xT, x16s=x16s):
                        tg = P["pg"].tile([128, 1024], F32, tag="pg")
                        for t in range(5):
                            nc.tensor.matmul(
                                tg[:, t * 128 : (t + 1) * 128],
                                lhsT=x16s[t][:, fc * 128 : (fc + 1) * 128],
                                rhs=ident16[:, 0:128],
                                start=True,
                                stop=True,
                            )
                        nc.vector.tensor_copy(xT[:, fc, :], tg[:, 0:640])
                    pieces.append(piece)
                return xT, pieces

            def emit_B(p, xT):
                """qk matmuls for pair p -> qTp (DMA) and kTpk (drain)."""
                sl = p % 2
                for fc in range(2 * KS):
                    tg = P["pg"].tile([128, 1024], F32, tag="pg")
                    for ks in range(KS):
                        nc.tensor.matmul(
                            tg[:, 0:512],
                            lhsT=wqkv16[:, ks, fc * 128 : (fc + 1) * 128],
                            rhs=xT[:, ks, 0:512],
                            start=(ks == 0),
                            stop=(ks == KS - 1),
                        )
                    for ks in range(KS):
                        nc.tensor.matmul(
                            tg[:, 512:640],
                            lhsT=wqkv16[:, ks, fc * 128 : (fc + 1) * 128],
                            rhs=xT[:, ks, 512:640],
                            start=(ks == 0),
                            stop=(ks == KS - 1),
                        )
                    if fc < KS:  # q features -> padded per-head tiles via DMA
                        qf = P["qkfc"].tile([128, PAIR_TOK], F16, tag="qkfc")
                        nc.vector.tensor_copy(qf[:], tg[:, 0:640])
                        nc.sync.dma_start(qTp[0:64, sl, 2 * fc, :], qf[0:64, :])
                        nc.sync.dma_start(qTp[64:128, sl, 2 * fc + 1, :], qf[64:128, :])
                    else:  # k features -> packed tile directly (alternate engines)
                        if fc % 2 == 0:
                            nc.vector.tensor_copy(kTpk[:, sl, fc - KS, :], tg[:, 0:640])
                        else:
                            nc.scalar.copy(kTpk[:, sl, fc - KS, :], tg[:, 0:640])

            def make_C(g):
                """v matmuls for batch g (3 psum tiles)."""
                sl, b2 = g % 2, g % 2
                btok = (g % 2) * N
                psl = (g // 2) % 2
                pieces = []
                for ci, (off, sz) in enumerate(KT_CHUNKS):
                    def piece(ci=ci, off=off, sz=sz):
                        xT = xT_cur[g // 2]
                        tg = P["pg"].tile([128, 1024], F32, tag="pg")
                        for ks in range(KS):
                            nc.tensor.matmul(
                                tg[:sz, 0:512],
                                lhsT=xT[:, ks, btok + off : btok + off + sz],
                                rhs=wqkv16[:, ks, 2 * C : 2 * C + 512],
                                start=(ks == 0),
                                stop=(ks == KS - 1),
                            )
                        for ks in range(KS):
                            nc.tensor.matmul(
                                tg[:sz, 512:768],
                                lhsT=xT[:, ks, btok + off : btok + off + sz],
                                rhs=wqkv16[:, ks, 2 * C + 512 : 3 * C],
                                start=(ks == 0),
                                stop=(ks == KS - 1),
                            )
                        nc.scalar.copy(
                            va[:sz, sl, ci, :, 0:64],
                            tg[:sz, 0:768].rearrange("p (h d) -> p h d", d=64),
                        )
                    pieces.append(piece)
                return pieces

            def emit_D(g, fillers):
                """Scores + exp for batch g, interleaving filler pieces."""
                sl = g % 2
                psl = (g // 2) % 2
                btok = (g % 2) * N
                fi = 0
                nf = len(fillers)
                gi = 0
                for ci, (koff, ksz) in enumerate(KT_CHUNKS):
                    for hg in range(3):
                        # two heads per matmul: both heads' padded q side by
                        # side (N=512); the packed kT chunk's parity halves
                        # each hit their own head, zeros kill cross terms
                        psc = P["ps"].tile([128, 2, 2, 256], F32, tag="ps")
                        for hp in range(2):
                            h0 = hg * 4 + hp * 2
                            nc.tensor.matmul(
                                psc[:ksz, hp, :, :],
                                lhsT=kTpk[:, psl, h0 // 2, btok + koff : btok + koff + ksz],
                                rhs=qTp[:, psl, h0 : h0 + 2, btok + 64 : btok + 320],
                                start=True,
                                stop=True,
                            )
                        nc.scalar.activation(
                            es[:ksz, sl, ci, hg * 4 : hg * 4 + 4, :],
                            psc[:ksz, :, :, :].rearrange("p a b q -> p (a b) q"),
                            mybir.ActivationFunctionType.Exp,
                            scale=0.125,
                        )
                        gi += 1
                        want = (nf * gi) // 9
                        while fi < want:
                            fillers[fi]()
                            fi += 1
                while fi < len(fillers):
                    fillers[fi]()
                    fi += 1

            def make_E(g):
                """Template scores + exp for batch g (2 pieces)."""
                sl = g % 2
                psl = (g // 2) % 2
                btok = (g % 2) * N
                holder = {}

                def mm_piece():
                    tg = P["pg"].tile([128, 1024], F32, tag="pg")
                    holder["tg"] = tg
                    for hp in range(6):
                        h0 = 2 * hp
                        nc.tensor.matmul(
                            tg[0:64, h0 * 64 : (h0 + 2) * 64],
                            lhsT=kTpk[:, psl, hp, btok : btok + 64],
                            rhs=qTp[:, psl, h0 : h0 + 2, btok : btok + 64],
                            start=True,
                            stop=True,
                        )

                def exp_piece():
                    tg = holder["tg"]
                    nc.scalar.activation(
                        esm[0:64, sl, :, :],
                        tg[0:64, 0:768].rearrange("p (h q) -> p h q", q=64),
                        mybir.ActivationFunctionType.Exp,
                        scale=0.125,
                    )

                return [mm_piece, exp_piece]

            def _normalize(tg, qsz, qg, half, sl):
                po_v = tg[:qsz, 0:510].rearrange("p (h s) -> p h s", s=SLOT)
                rcp = P["rcp"].tile([128, 8], F32, tag="rcp")
                nc.vector.reciprocal(rcp[:qsz, 0:6], po_v[:, :, 64])
                nc.vector.tensor_tensor(
                    attn[:qsz, sl, qg, half * 384 : (half + 1) * 384].rearrange(
                        "p (h d) -> p h d", d=64
                    ),
                    po_v[:, :, 0:64],
                    rcp[:qsz, 0:6, None].to_broadcast([qsz, 6, 64]),
                    mybir.AluOpType.mult,
                )

            def make_FGH(g):
                """PV + normalize + attn^T + proj for batch g (deferred)."""
                sl = g % 2
                pieces = []

                # template PV (2 pieces, one per head-half)
                tpv_pieces = []
                for half in range(2):
                    def tpv(half=half):
                        tg = P["pg"].tile([128, 1024], F32, tag="pg")
                        for j in range(6):
                            h = half * 6 + j
                            nc.tensor.matmul(
                                tg[0:64, j * SLOT : j * SLOT + 65],
                                lhsT=esm[:, sl, h, 0:64],
                                rhs=va[:, sl, 0, h, 0:65],
                                start=True,
                                stop=True,
                            )
                        _normalize(tg, 64, 0, half, sl)
                    tpv_pieces.append(tpv)

                # search PV (4 pieces: qg x half)
                spv_pieces = {}
                for qg in (1, 2):
                    for half in range(2):
                        def spv(qg=qg, half=half):
                            tg = P["pg"].tile([128, 1024], F32, tag="pg")
                            for j in range(6):
                                h = half * 6 + j
                                for ci in range(3):
                                    nc.tensor.matmul(
                                        tg[0:128, j * SLOT : j * SLOT + 65],
                                        lhsT=es[:, sl, ci, h, (qg - 1) * 128 : qg * 128],
                                        rhs=va[:, sl, ci, h, 0:65],
                                        start=(ci == 0),
                                        stop=(ci == 2),
                                    )
                            _normalize(tg, 128, qg, half, sl)
                        spv_pieces[(qg, half)] = spv

                # attn^T via regular matmuls (6 pieces)
                at_pieces = []
                for fc in range(KS):
                    def at(fc=fc):
                        tg = P["pg"].tile([128, 1024], F32, tag="pg")
                        # overlap-packed: qg0 -> 0:128 (real 0:64), qg1 -> 64:192,
                        # qg2 -> 192:320
                        for qg, dst0 in ((0, 0), (1, 64), (2, 192)):
                            nc.tensor.matmul(
                                tg[:, dst0 : dst0 + 128],
                                lhsT=attn[0:128, sl, qg, fc * 128 : (fc + 1) * 128],
                                rhs=ident16[:, 0:128],
                                start=True,
                                stop=True,
                            )
                        nc.vector.tensor_copy(attnT[:, sl, fc, 0:N], tg[:, 0:N])
                    at_pieces.append(at)

                # proj + bias + out DMA (3 pieces)
                pieces = []
                row0 = g * N
                for qc, (qoff, qsz) in enumerate(P_CHUNKS):
                    def pj(qc=qc, qoff=qoff, qsz=qsz):
                        tg = P["pg"].tile([128, 1024], F32, tag="pg")
                        for ks in range(KS):
                            nc.tensor.matmul(
                                tg[:qsz, 0:512],
                                lhsT=attnT[:, sl, ks, qoff : qoff + qsz],
                                rhs=wproj16[:, ks, 0:512],
                                start=(ks == 0),
                                stop=(ks == KS - 1),
                            )
                        for ks in range(KS):
                            nc.tensor.matmul(
                                tg[:qsz, 512:768],
                                lhsT=attnT[:, sl, ks, qoff : qoff + qsz],
                                rhs=wproj16[:, ks, 512:768],
                                start=(ks == 0),
                                stop=(ks == KS - 1),
                            )
                        ost = P["outst"].tile([128, C], F32, tag="outst")
                        nc.vector.tensor_tensor(
                            ost[:qsz, :], tg[:qsz, 0:768], bias_bc[:qsz, :],
                            mybir.AluOpType.add,
                        )
                        nc.sync.dma_start(
                            out_ap[row0 + qoff : row0 + qoff + qsz, :], ost[:qsz, :]
                        )
                    pieces.append(pj)
                pj_pieces = pieces
                # order: half-0 PV -> attnT fc 0-2 -> half-1 PV -> attnT 3-5
                # -> proj; gets attn^T/proj flowing as early as possible
                return (
                    [tpv_pieces[0], spv_pieces[(1, 0)], spv_pieces[(2, 0)]]
                    + at_pieces[0:3]
                    + [tpv_pieces[1], spv_pieces[(1, 1)], spv_pieces[(2, 1)]]
                    + at_pieces[3:6]
                    + pj_pieces
                )

            # ================= main schedule =================
            # x DMAs first (unblock PE transposes ASAP), then the weight
            # block (HBM-bound), pads last so DVE drains xT promptly.
            xT_cur = {}
            xtiles = emit_xdma(0)
            xT0, a_pieces = emit_A(0, xtiles)
            emit_weight_load()
            for pc in a_pieces:
                pc()
            xT_cur[0] = xT0
            emit_pads()

            stash = []
            for p in range(NPAIR):
                for pc in stash:  # F/G/H of batch 2p-1
                    pc()
                stash = []
                if p + 1 < NPAIR:
                    xtiles = emit_xdma(p + 1)
                emit_B(p, xT_cur[p])
                g0, g1 = 2 * p, 2 * p + 1
                for pc in make_C(g0):
                    pc()
                fill0 = make_C(g1) + make_E(g0)
                if p + 1 < NPAIR:
                    xT1, a_next = emit_A(p + 1, xtiles)
                    xT_cur[p + 1] = xT1
                    fill0 = fill0 + a_next
                emit_D(g0, fill0)
                stash0 = make_FGH(g0)
                emit_D(g1, stash0 + make_E(g1))
                stash = make_FGH(g1)
            for pc in stash:
                pc()

    nc.compile()
    return nc


@functools.cache
def _get_nc():
    return build_kernel()


def make_in_maps(x, wqkv, wproj, bias):
    x16 = np.ascontiguousarray(x.reshape(B, N, C).astype(np.float16))
    wqkv16 = np.ascontiguousarray(wqkv.astype(np.float16))
    wproj16 = np.ascontiguousarray(wproj.astype(np.float16))
    bias = np.ascontiguousarray(bias.astype(np.float32))
    return [
        {
            "x16": np.ascontiguousarray(
                x16[c * B_CORE : (c + 1) * B_CORE].reshape(TOK_CORE, C)
            ),
            "W_qkv16": wqkv16,
            "W_proj16": wproj16,
            "b_proj": bias,
        }
        for c in range(NCORES)
    ]


def kernel(**inputs):
    x = np.ascontiguousarray(np.asarray(inputs["x"], dtype=np.float32))
    wqkv = np.ascontiguousarray(np.asarray(inputs["W_qkv"], dtype=np.float32))
    wproj = np.ascontiguousarray(np.asarray(inputs["W_proj"], dtype=np.float32))
    bias = np.ascontiguousarray(np.asarray(inputs["b_proj"], dtype=np.float32))
    t_h = int(inputs.get("t_h", 8))
    t_w = int(inputs.get("t_w", 8))
    assert t_h * t_w == 64, "kernel built for template length 64"
    assert x.shape == (B, N, C)

    nc = _get_nc()
    in_maps = make_in_maps(x, wqkv, wproj, bias)
    res = run_bass_kernel_spmd(nc, in_maps, core_ids=list(range(NCORES)))
    out = np.concatenate(
        [r["out"].reshape(B_CORE, N, C) for r in res.results], axis=0
    )
    return out.astype(np.float32)


if __name__ == "__main__":
    _get_nc()
    print("kernel_v2 built OK")


# revision 5
# speedup vs baseline: 1.1623x; 1.0330x over previous
"""Trainium2 Bass kernel for MixformerAttention (sparse attention) — v2.

Problem shape (hardcoded):
  x [B=64, N=320, C=768], W_qkv [768, 2304], W_proj [768, 768], b_proj [768]
  H=12 heads, Dh=64, template L=64, search=256. DP over batch on 8 cores.

v2 redesign vs baseline (empirically driven by the NTFF trace):
  * Every matmul whose lhsT had only 64 partition rows (scores, template,
    PV tail) paid a ~100ns serialized LDWEIGHTS. All attention operands are
    now zero-padded to full 128 contraction rows:
      - qTp: per-head q tiles [128, tok], data in the head's native 64-row
        half, zeros in the other half (DMA'd from the qk psum drain).
      - kT stays packed 2-heads/chunk; the junk half multiplies the zero
        half of qTp, contributing 0.
      - es/esm/va key-padded the same way (persistent tiles, pads zeroed
        once at startup).
  * attn^T computed with regular matmuls (lhsT=attn chunk, rhs=identity):
    weight-load overlaps, unlike transpose-mode where the data IS the
    weight load. Template/search chunks overlap-packed into one psum strip
    so each fc drains with a single copy.
  * Software-pipelined emission: scores/exp of batch g interleave with
    PV/attnT/proj of batch g-1 (deferred thunks), so the Act-engine exp
    latency never stalls the PE.
  * PSUM budget exactly 8 banks: pool_g 3x[128,768] + pool_s 1x[128,1024].
"""

import contextlib
import functools

import numpy as np

import concourse.bacc as bacc
import concourse.mybir as mybir
from concourse.bass_utils import run_bass_kernel_spmd
from concourse.masks import make_identity
from concourse.tile import TileContext

F32 = mybir.dt.float32
F16 = mybir.dt.float16

NCORES = 8
B, N, C = 64, 320, 768
H, DH = 12, 64
KS = C // 128  # 6 contraction subtiles
B_CORE = B // NCORES  # 8
PAIR_TOK = 2 * N  # 640
NPAIR = B_CORE // 2  # 4
TOK_CORE = B_CORE * N  # 2560
SLOT = 85  # psum col stride per head in PV output (6 heads in 510 cols)

KT_CHUNKS = [(0, 128), (128, 128), (256, 64)]  # key chunks per batch
P_CHUNKS = [(0, 128), (128, 128), (256, 64)]  # proj token chunks per batch


def build_kernel():
    nc = bacc.Bacc("TRN2", target_bir_lowering=False)
    x_t = nc.dram_tensor("xT16", [C, TOK_CORE], F16, kind="ExternalInput")
    wqkv_t = nc.dram_tensor("W_qkv16", [C, 3 * C], F16, kind="ExternalInput")
    wproj_t = nc.dram_tensor("W_proj16", [C, C], F16, kind="ExternalInput")
    bias_t = nc.dram_tensor("b_proj", [C], F32, kind="ExternalInput")
    out_t = nc.dram_tensor("out", [TOK_CORE, C], F32, kind="ExternalOutput")
    x_ap, out_ap = x_t.ap(), out_t.ap()

    with TileContext(nc) as tc:
        with contextlib.ExitStack() as ctx:
            P = {
                "const": ctx.enter_context(tc.tile_pool(name="const", bufs=1)),
                "stagep": ctx.enter_context(tc.tile_pool(name="stagep", bufs=1)),
                "xT": ctx.enter_context(tc.tile_pool(name="xT", bufs=2)),
                "qkfc": ctx.enter_context(tc.tile_pool(name="qkfc", bufs=3)),
                "outst": ctx.enter_context(tc.tile_pool(name="outst", bufs=2)),
                "rcp": ctx.enter_context(tc.tile_pool(name="rcp", bufs=4)),
                "pg": ctx.enter_context(tc.tile_pool(name="pg", bufs=3, space="PSUM")),
                "ps": ctx.enter_context(tc.tile_pool(name="ps", bufs=1, space="PSUM")),
            }
            const = P["const"]

            # ---- persistent constants ----
            wqkv16 = const.tile([128, KS, 3 * C], F16, tag="wqkv16")
            wproj16 = const.tile([128, KS, C], F16, tag="wproj16")
            bias_bc = const.tile([128, C], F32, tag="bias_bc")
            ident32 = const.tile([128, 128], F32, tag="ident32")
            ident16 = const.tile([128, 128], F16, tag="ident16")
            make_identity(nc, ident32)
            make_identity(nc, ident16)

            # ---- persistent double-slotted activation tiles ----
            # per-head padded q (slot = pair parity)
            qTp = const.tile([128, 2, H, PAIR_TOK], F16, tag="qTp")
            # packed kT feature chunks (2 heads per chunk)
            kTpk = const.tile([128, 2, KS, PAIR_TOK], F16, tag="kTpk")
            # v natural with ones column (slot = batch parity)
            va = const.tile([128, 2, 3, H, 66], F16, tag="va")
            # exp(scores) for search queries [key, h, q]  (256 = search q)
            es = const.tile([128, 2, 3, H, 256], F16, tag="es")
            # exp(scores) template [key<=64 padded, h, q0:64]
            esm = const.tile([128, 2, H, 64], F16, tag="esm")
            # attention rows (template 64 padded | search 128 | search 128)
            attn = const.tile([128, 2, 3, C], F16, tag="attn")
            # attn^T per batch [C-part, tok]
            attnT = const.tile([128, 2, KS, N], F16, tag="attnT")

            def emit_weight_load():
                # fp16 weights land directly in their SBUF tiles (host-cast)
                for ks in range(KS):
                    nc.sync.dma_start(
                        wqkv16[:, ks, :], wqkv_t.ap()[ks * 128 : (ks + 1) * 128, :]
                    )
                for ks in range(KS):
                    nc.sync.dma_start(
                        wproj16[:, ks, :], wproj_t.ap()[ks * 128 : (ks + 1) * 128, :]
                    )
                brow = P["stagep"].tile([128, C], F32, tag="stagep")
                nc.sync.dma_start(brow[0:1, 0:C], bias_t.ap().unsqueeze(0))
                nc.gpsimd.partition_broadcast(bias_bc[:, :], brow[0:1, 0:C])

            def emit_pads():
                # one-time pad zeroing (emitted after A(0) so the DVE queue
                # drains xT first; WAW deps keep correctness)
                nc.gpsimd.memset(qTp[64:128, :, 0:H:2, :], 0.0)  # even heads
                nc.vector.memset(qTp[0:64, :, 1:H:2, :], 0.0)  # odd heads
                nc.vector.memset(es[64:128, :, 2, :, :], 0.0)  # key chunk 2
                nc.vector.memset(esm[64:128, :, :, :], 0.0)  # template keys
                nc.vector.memset(attn[64:128, :, 0, :], 0.0)  # template rows
                nc.gpsimd.memset(va[64:128, :, 2, :, :], 0.0)  # v key chunk 2
                nc.vector.memset(va[:, :, :, :, 64], 1.0)  # ones column
                nc.vector.memset(va[:, :, :, :, 65], 0.0)  # stride pad

            # ================= emission helpers =================

            def emit_xT(p):
                """DMA the pre-transposed x^T slab of pair p into SBUF."""
                xT = P["xT"].tile([128, KS, PAIR_TOK], F16, tag="xT")
                nc.sync.dma_start(
                    xT[:],
                    x_ap[:, p * PAIR_TOK : (p + 1) * PAIR_TOK].rearrange(
                        "(k p) t -> p k t", p=128
                    ),
                )
                return xT

            def emit_B(p, xT):
                """qk matmuls for pair p -> qTp (DMA) and kTpk (drain)."""
                sl = p % 2
                for fc in [0, 6, 1, 7, 2, 8, 3, 9, 4, 10, 5, 11]:
                    tg = P["pg"].tile([128, 1024], F32, tag="pg")
                    for ks in range(KS):
                        nc.tensor.matmul(
                            tg[:, 0:512],
                            lhsT=wqkv16[:, ks, fc * 128 : (fc + 1) * 128],
                            rhs=xT[:, ks, 0:512],
                            start=(ks == 0),
                            stop=(ks == KS - 1),
                        )
                    for ks in range(KS):
                        nc.tensor.matmul(
                            tg[:, 512:640],
                            lhsT=wqkv16[:, ks, fc * 128 : (fc + 1) * 128],
                            rhs=xT[:, ks, 512:640],
                            start=(ks == 0),
                            stop=(ks == KS - 1),
                        )
                    if fc < KS:  # q features -> padded per-head tiles via DMA
                        qf = P["qkfc"].tile([128, PAIR_TOK], F16, tag="qkfc")
                        nc.vector.tensor_copy(qf[:], tg[:, 0:640])
                        nc.sync.dma_start(qTp[0:64, sl, 2 * fc, :], qf[0:64, :])
                        nc.sync.dma_start(qTp[64:128, sl, 2 * fc + 1, :], qf[64:128, :])
                    else:  # k features -> packed tile directly (alternate engines)
                        if fc % 2 == 0:
                            nc.vector.tensor_copy(kTpk[:, sl, fc - KS, :], tg[:, 0:640])
                        else:
                            nc.scalar.copy(kTpk[:, sl, fc - KS, :], tg[:, 0:640])

            def make_C(g):
                """v matmuls for batch g (3 psum tiles)."""
                sl, b2 = g % 2, g % 2
                btok = (g % 2) * N
                psl = (g // 2) % 2
                pieces = []
                for ci, (off, sz) in enumerate(KT_CHUNKS):
                    holder = {}

                    def piece_a(ci=ci, off=off, sz=sz, holder=holder):
                        xT = xT_cur[g // 2]
                        tg = P["pg"].tile([128, 1024], F32, tag="pg")
                        holder["tg"] = tg
                        for ks in range(KS):
                            nc.tensor.matmul(
                                tg[:sz, 0:512],
                                lhsT=xT[:, ks, btok + off : btok + off + sz],
                                rhs=wqkv16[:, ks, 2 * C : 2 * C + 512],
                                start=(ks == 0),
                                stop=(ks == KS - 1),
                            )

                    def piece_b(ci=ci, off=off, sz=sz, holder=holder):
                        xT = xT_cur[g // 2]
                        tg = holder["tg"]
                        for ks in range(KS):
                            nc.tensor.matmul(
                                tg[:sz, 512:768],
                                lhsT=xT[:, ks, btok + off : btok + off + sz],
                                rhs=wqkv16[:, ks, 2 * C + 512 : 3 * C],
                                start=(ks == 0),
                                stop=(ks == KS - 1),
                            )
                        nc.scalar.copy(
                            va[:sz, sl, ci, :, 0:64],
                            tg[:sz, 0:768].rearrange("p (h d) -> p h d", d=64),
                        )

                    pieces.append(piece_a)
                    pieces.append(piece_b)
                return pieces

            def emit_D(g, fillers):
                """Scores + exp for batch g, interleaving filler pieces."""
                sl = g % 2
                psl = (g // 2) % 2
                btok = (g % 2) * N
                fi = 0
                nf = len(fillers)
                gi = 0
                for ci, (koff, ksz) in enumerate(KT_CHUNKS):
                    for hg in range(3):
                        # two heads per matmul: both heads' padded q side by
                        # side (N=512); the packed kT chunk's parity halves
                        # each hit their own head, zeros kill cross terms
                        psc = P["ps"].tile([128, 2, 2, 256], F32, tag="ps")
                        for hp in range(2):
                            h0 = hg * 4 + hp * 2
                            nc.tensor.matmul(
                                psc[:ksz, hp, :, :],
                                lhsT=kTpk[:, psl, h0 // 2, btok + koff : btok + koff + ksz],
                                rhs=qTp[:, psl, h0 : h0 + 2, btok + 64 : btok + 320],
                                start=True,
                                stop=True,
                            )
                        nc.scalar.activation(
                            es[:ksz, sl, ci, hg * 4 : hg * 4 + 4, :],
                            psc[:ksz, :, :, :].rearrange("p a b q -> p (a b) q"),
                            mybir.ActivationFunctionType.Exp,
                            scale=0.125,
                        )
                        gi += 1
                        want = (nf * gi) // 9
                        while fi < want:
                            fillers[fi]()
                            fi += 1
                while fi < len(fillers):
                    fillers[fi]()
                    fi += 1

            def make_E(g):
                """Template scores + exp for batch g (2 pieces)."""
                sl = g % 2
                psl = (g // 2) % 2
                btok = (g % 2) * N
                holder = {}

                def mm_piece():
                    tg = P["pg"].tile([128, 1024], F32, tag="pg")
                    holder["tg"] = tg
                    for hp in range(6):
                        h0 = 2 * hp
                        nc.tensor.matmul(
                            tg[0:128, h0 * 64 : (h0 + 2) * 64],
                            lhsT=kTpk[:, psl, hp, btok : btok + 128],
                            rhs=qTp[:, psl, h0 : h0 + 2, btok : btok + 64],
                            start=True,
                            stop=True,
                        )

                def exp_piece():
                    tg = holder["tg"]
                    nc.scalar.activation(
                        esm[0:64, sl, :, :],
                        tg[0:64, 0:768].rearrange("p (h q) -> p h q", q=64),
                        mybir.ActivationFunctionType.Exp,
                        scale=0.125,
                    )

                return [mm_piece, exp_piece]

            def _normalize(tg, qsz, qg, half, sl):
                po_v = tg[:qsz, 0:510].rearrange("p (h s) -> p h s", s=SLOT)
                rcp = P["rcp"].tile([128, 8], F32, tag="rcp")
                nc.vector.reciprocal(rcp[:qsz, 0:6], po_v[:, :, 64])
                nc.vector.tensor_tensor(
                    attn[:qsz, sl, qg, half * 384 : (half + 1) * 384].rearrange(
                        "p (h d) -> p h d", d=64
                    ),
                    po_v[:, :, 0:64],
                    rcp[:qsz, 0:6, None].to_broadcast([qsz, 6, 64]),
                    mybir.AluOpType.mult,
                )

            def make_FGH(g):
                """PV + normalize + attn^T + proj for batch g (deferred)."""
                sl = g % 2
                pieces = []

                # template PV (2 pieces, one per head-half)
                tpv_pieces = []
                for half in range(2):
                    def tpv(half=half):
                        tg = P["pg"].tile([128, 1024], F32, tag="pg")
                        for j in range(6):
                            h = half * 6 + j
                            nc.tensor.matmul(
                                tg[0:64, j * SLOT : j * SLOT + 65],
                                lhsT=esm[:, sl, h, 0:64],
                                rhs=va[:, sl, 0, h, 0:65],
                                start=True,
                                stop=True,
                            )
                        _normalize(tg, 64, 0, half, sl)
                    tpv_pieces.append(tpv)

                # search PV (4 pieces: qg x half)
                spv_pieces = {}
                for qg in (1, 2):
                    for half in range(2):
                        def spv(qg=qg, half=half):
                            tg = P["pg"].tile([128, 1024], F32, tag="pg")
                            for j in range(6):
                                h = half * 6 + j
                                for ci in range(3):
                                    nc.tensor.matmul(
                                        tg[0:128, j * SLOT : j * SLOT + 65],
                                        lhsT=es[:, sl, ci, h, (qg - 1) * 128 : qg * 128],
                                        rhs=va[:, sl, ci, h, 0:65],
                                        start=(ci == 0),
                                        stop=(ci == 2),
                                    )
                            _normalize(tg, 128, qg, half, sl)
                        spv_pieces[(qg, half)] = spv

                # attn^T via regular matmuls (6 pieces)
                at_pieces = []
                for fc in range(KS):
                    def at(fc=fc):
                        tg = P["pg"].tile([128, 1024], F32, tag="pg")
                        # overlap-packed: qg0 -> 0:128 (real 0:64), qg1 -> 64:192,
                        # qg2 -> 192:320
                        for qg, dst0 in ((0, 0), (1, 64), (2, 192)):
                            nc.tensor.matmul(
                                tg[:, dst0 : dst0 + 128],
                                lhsT=attn[0:128, sl, qg, fc * 128 : (fc + 1) * 128],
                                rhs=ident16[:, 0:128],
                                start=True,
                                stop=True,
                            )
                        nc.vector.tensor_copy(attnT[:, sl, fc, 0:N], tg[:, 0:N])
                    at_pieces.append(at)

                # proj + bias + out DMA (3 pieces)
                pieces = []
                row0 = g * N
                for qc, (qoff, qsz) in enumerate(P_CHUNKS):
                    def pj(qc=qc, qoff=qoff, qsz=qsz):
                        tg = P["pg"].tile([128, 1024], F32, tag="pg")
                        for ks in range(KS):
                            nc.tensor.matmul(
                                tg[:qsz, 0:512],
                                lhsT=attnT[:, sl, ks, qoff : qoff + qsz],
                                rhs=wproj16[:, ks, 0:512],
                                start=(ks == 0),
                                stop=(ks == KS - 1),
                            )
                        for ks in range(KS):
                            nc.tensor.matmul(
                                tg[:qsz, 512:768],
                                lhsT=attnT[:, sl, ks, qoff : qoff + qsz],
                                rhs=wproj16[:, ks, 512:768],
                                start=(ks == 0),
                                stop=(ks == KS - 1),
                            )
                        ost = P["outst"].tile([128, C], F32, tag="outst")
                        nc.vector.tensor_tensor(
                            ost[:qsz, :], tg[:qsz, 0:768], bias_bc[:qsz, :],
                            mybir.AluOpType.add,
                        )
                        nc.sync.dma_start(
                            out_ap[row0 + qoff : row0 + qoff + qsz, :], ost[:qsz, :]
                        )
                    pieces.append(pj)
                pj_pieces = pieces
                # order: half-0 PV -> attnT fc 0-2 -> half-1 PV -> attnT 3-5
                # -> proj; gets attn^T/proj flowing as early as possible
                return (
                    [tpv_pieces[0], spv_pieces[(1, 0)], spv_pieces[(2, 0)]]
                    + at_pieces[0:3]
                    + [tpv_pieces[1], spv_pieces[(1, 1)], spv_pieces[(2, 1)]]
                    + at_pieces[3:6]
                    + pj_pieces
                )

            # ================= main schedule =================
            # x DMAs first (unblock PE transposes ASAP), then the weight
            # block (HBM-bound), pads last so DVE drains xT promptly.
            xT_cur = {}
            xT_cur[0] = emit_xT(0)
            emit_weight_load()
            emit_pads()

            stash = []
            for p in range(NPAIR):
                for pc in stash:  # F/G/H of batch 2p-1
                    pc()
                stash = []
                if p + 1 < NPAIR:
                    xT_cur[p + 1] = emit_xT(p + 1)
                emit_B(p, xT_cur[p])
                g0, g1 = 2 * p, 2 * p + 1
                for pc in make_C(g0):
                    pc()
                fill0 = make_C(g1) + make_E(g0)
                emit_D(g0, fill0)
                stash0 = make_FGH(g0)
                emit_D(g1, stash0 + make_E(g1))
                stash = make_FGH(g1)
            for pc in stash:
                pc()

    nc.compile()
    return nc


@functools.cache
def _get_nc():
    return build_kernel()


def make_in_maps(x, wqkv, wproj, bias):
    x16 = x.reshape(B, N, C).astype(np.float16)
    wqkv16 = np.ascontiguousarray(wqkv.astype(np.float16))
    wproj16 = np.ascontiguousarray(wproj.astype(np.float16))
    bias = np.ascontiguousarray(bias.astype(np.float32))
    return [
        {
            "xT16": np.ascontiguousarray(
                x16[c * B_CORE : (c + 1) * B_CORE].reshape(TOK_CORE, C).T
            ),
            "W_qkv16": wqkv16,
            "W_proj16": wproj16,
            "b_proj": bias,
        }
        for c in range(NCORES)
    ]


def kernel(**inputs):
    x = np.ascontiguousarray(np.asarray(inputs["x"], dtype=np.float32))
    wqkv = np.ascontiguousarray(np.asarray(inputs["W_qkv"], dtype=np.float32))
    wproj = np.ascontiguousarray(np.asarray(inputs["W_proj"], dtype=np.float32))
    bias = np.ascontiguousarray(np.asarray(inputs["b_proj"], dtype=np.float32))
    t_h = int(inputs.get("t_h", 8))
    t_w = int(inputs.get("t_w", 8))
    assert t_h * t_w == 64, "kernel built for template length 64"
    assert x.shape == (B, N, C)

    nc = _get_nc()
    in_maps = make_in_maps(x, wqkv, wproj, bias)
    res = run_bass_kernel_spmd(nc, in_maps, core_ids=list(range(NCORES)))
    out = np.concatenate(
        [r["out"].reshape(B_CORE, N, C) for r in res.results], axis=0
    )
    return out.astype(np.float32)


if __name__ == "__main__":
    _get_nc()
    print("kernel_v2 built OK")


# revision 6
# speedup vs baseline: 1.2024x; 1.0345x over previous
"""Trainium2 Bass kernel for MixformerAttention (sparse attention) — v2.

Problem shape (hardcoded):
  x [B=64, N=320, C=768], W_qkv [768, 2304], W_proj [768, 768], b_proj [768]
  H=12 heads, Dh=64, template L=64, search=256. DP over batch on 8 cores.

v2 redesign vs baseline (empirically driven by the NTFF trace):
  * Every matmul whose lhsT had only 64 partition rows (scores, template,
    PV tail) paid a ~100ns serialized LDWEIGHTS. All attention operands are
    now zero-padded to full 128 contraction rows:
      - qTp: per-head q tiles [128, tok], data in the head's native 64-row
        half, zeros in the other half (DMA'd from the qk psum drain).
      - kT stays packed 2-heads/chunk; the junk half multiplies the zero
        half of qTp, contributing 0.
      - es/esm/va key-padded the same way (persistent tiles, pads zeroed
        once at startup).
  * attn^T computed with regular matmuls (lhsT=attn chunk, rhs=identity):
    weight-load overlaps, unlike transpose-mode where the data IS the
    weight load. Template/search chunks overlap-packed into one psum strip
    so each fc drains with a single copy.
  * Software-pipelined emission: scores/exp of batch g interleave with
    PV/attnT/proj of batch g-1 (deferred thunks), so the Act-engine exp
    latency never stalls the PE.
  * PSUM budget exactly 8 banks: pool_g 3x[128,768] + pool_s 1x[128,1024].
"""

import contextlib
import functools

import numpy as np

import concourse.bacc as bacc
import concourse.mybir as mybir
from concourse.bass_utils import run_bass_kernel_spmd
from concourse.masks import make_identity
from concourse.tile import TileContext

F32 = mybir.dt.float32
F16 = mybir.dt.float16

NCORES = 8
B, N, C = 64, 320, 768
H, DH = 12, 64
KS = C // 128  # 6 contraction subtiles
B_CORE = B // NCORES  # 8
PAIR_TOK = 2 * N  # 640
NPAIR = B_CORE // 2  # 4
TOK_CORE = B_CORE * N  # 2560
SLOT = 85  # psum col stride per head in PV output (6 heads in 510 cols)

KT_CHUNKS = [(0, 128), (128, 128), (256, 64)]  # key chunks per batch
P_CHUNKS = [(0, 128), (128, 128), (256, 64)]  # proj token chunks per batch


def build_kernel():
    nc = bacc.Bacc("TRN2", target_bir_lowering=False)
    x_t = nc.dram_tensor("xT16", [C, TOK_CORE], F16, kind="ExternalInput")
    wqkv_t = nc.dram_tensor("W_qkv16", [C, 3 * C], F16, kind="ExternalInput")
    wproj_t = nc.dram_tensor("W_proj16", [C, C], F16, kind="ExternalInput")
    bias_t = nc.dram_tensor("b_proj", [C], F32, kind="ExternalInput")
    out_t = nc.dram_tensor("out", [TOK_CORE, C], F16, kind="ExternalOutput")
    x_ap, out_ap = x_t.ap(), out_t.ap()

    with TileContext(nc) as tc:
        with contextlib.ExitStack() as ctx:
            P = {
                "const": ctx.enter_context(tc.tile_pool(name="const", bufs=1)),
                "stagep": ctx.enter_context(tc.tile_pool(name="stagep", bufs=1)),
                "xT": ctx.enter_context(tc.tile_pool(name="xT", bufs=2)),
                "qkfc": ctx.enter_context(tc.tile_pool(name="qkfc", bufs=3)),
                "outst": ctx.enter_context(tc.tile_pool(name="outst", bufs=2)),
                "rcp": ctx.enter_context(tc.tile_pool(name="rcp", bufs=4)),
                "pg": ctx.enter_context(tc.tile_pool(name="pg", bufs=2, space="PSUM")),
                "ps": ctx.enter_context(tc.tile_pool(name="ps", bufs=2, space="PSUM")),
            }
            const = P["const"]

            # ---- persistent constants ----
            wqkv16 = const.tile([128, KS, 3 * C], F16, tag="wqkv16")
            wproj16 = const.tile([128, KS, C], F16, tag="wproj16")
            bias_bc = const.tile([128, C], F32, tag="bias_bc")
            ident32 = const.tile([128, 128], F32, tag="ident32")
            ident16 = const.tile([128, 128], F16, tag="ident16")
            make_identity(nc, ident32)
            make_identity(nc, ident16)

            # ---- persistent double-slotted activation tiles ----
            # per-head padded q (slot = pair parity)
            qTp = const.tile([128, 2, H, PAIR_TOK], F16, tag="qTp")
            # packed kT feature chunks (2 heads per chunk)
            kTpk = const.tile([128, 2, KS, PAIR_TOK], F16, tag="kTpk")
            # v natural with ones column (slot = batch parity)
            va = const.tile([128, 2, 3, H, 66], F16, tag="va")
            # exp(scores) for search queries [key, h, q]  (256 = search q)
            es = const.tile([128, 2, 3, H, 256], F16, tag="es")
            # exp(scores) template [key<=64 padded, h, q0:64]
            esm = const.tile([128, 2, H, 64], F16, tag="esm")
            # attention rows (template 64 padded | search 128 | search 128)
            attn = const.tile([128, 2, 3, C], F16, tag="attn")
            # attn^T per batch [C-part, tok]
            attnT = const.tile([128, 2, KS, N], F16, tag="attnT")

            def emit_weight_load():
                # fp16 weights land directly in their SBUF tiles (host-cast)
                for ks in range(KS):
                    nc.sync.dma_start(
                        wqkv16[:, ks, :], wqkv_t.ap()[ks * 128 : (ks + 1) * 128, :]
                    )
                for ks in range(KS):
                    nc.sync.dma_start(
                        wproj16[:, ks, :], wproj_t.ap()[ks * 128 : (ks + 1) * 128, :]
                    )
                brow = P["stagep"].tile([128, C], F32, tag="stagep")
                nc.sync.dma_start(brow[0:1, 0:C], bias_t.ap().unsqueeze(0))
                nc.gpsimd.partition_broadcast(bias_bc[:, :], brow[0:1, 0:C])

            def emit_pads():
                # one-time pad zeroing (emitted after A(0) so the DVE queue
                # drains xT first; WAW deps keep correctness)
                nc.gpsimd.memset(qTp[64:128, :, 0:H:2, :], 0.0)  # even heads
                nc.vector.memset(qTp[0:64, :, 1:H:2, :], 0.0)  # odd heads
                nc.vector.memset(es[64:128, :, 2, :, :], 0.0)  # key chunk 2
                nc.vector.memset(esm[64:128, :, :, :], 0.0)  # template keys
                nc.vector.memset(attn[64:128, :, 0, :], 0.0)  # template rows
                nc.gpsimd.memset(va[64:128, :, 2, :, :], 0.0)  # v key chunk 2
                nc.vector.memset(va[:, :, :, :, 64], 1.0)  # ones column
                nc.vector.memset(va[:, :, :, :, 65], 0.0)  # stride pad

            # ================= emission helpers =================

            def emit_xT(p):
                """DMA the pre-transposed x^T slab of pair p into SBUF."""
                xT = P["xT"].tile([128, KS, PAIR_TOK], F16, tag="xT")
                nc.sync.dma_start(
                    xT[:],
                    x_ap[:, p * PAIR_TOK : (p + 1) * PAIR_TOK].rearrange(
                        "(k p) t -> p k t", p=128
                    ),
                )
                return xT

            def emit_B(p, xT):
                """qk matmuls for pair p -> qTp (DMA) and kTpk (drain)."""
                sl = p % 2
                for fc in [0, 6, 1, 7, 2, 8, 3, 9, 4, 10, 5, 11]:
                    tg = P["pg"].tile([128, 1024], F32, tag="pg")
                    for ks in range(KS):
                        nc.tensor.matmul(
                            tg[:, 0:512],
                            lhsT=wqkv16[:, ks, fc * 128 : (fc + 1) * 128],
                            rhs=xT[:, ks, 0:512],
                            start=(ks == 0),
                            stop=(ks == KS - 1),
                        )
                    for ks in range(KS):
                        nc.tensor.matmul(
                            tg[:, 512:640],
                            lhsT=wqkv16[:, ks, fc * 128 : (fc + 1) * 128],
                            rhs=xT[:, ks, 512:640],
                            start=(ks == 0),
                            stop=(ks == KS - 1),
                        )
                    if fc < KS:  # q features -> padded per-head tiles via DMA
                        qf = P["qkfc"].tile([128, PAIR_TOK], F16, tag="qkfc")
                        nc.vector.tensor_copy(qf[:], tg[:, 0:640])
                        nc.sync.dma_start(qTp[0:64, sl, 2 * fc, :], qf[0:64, :])
                        nc.sync.dma_start(qTp[64:128, sl, 2 * fc + 1, :], qf[64:128, :])
                    else:  # k features -> packed tile directly (alternate engines)
                        if fc % 2 == 0:
                            nc.vector.tensor_copy(kTpk[:, sl, fc - KS, :], tg[:, 0:640])
                        else:
                            nc.scalar.copy(kTpk[:, sl, fc - KS, :], tg[:, 0:640])

            def make_C(g):
                """v matmuls for batch g (3 psum tiles)."""
                sl, b2 = g % 2, g % 2
                btok = (g % 2) * N
                psl = (g // 2) % 2
                pieces = []
                for ci, (off, sz) in enumerate(KT_CHUNKS):
                    holder = {}

                    def piece_a(ci=ci, off=off, sz=sz, holder=holder):
                        xT = xT_cur[g // 2]
                        tg = P["pg"].tile([128, 1024], F32, tag="pg")
                        holder["tg"] = tg
                        for ks in range(KS):
                            nc.tensor.matmul(
                                tg[:sz, 0:512],
                                lhsT=xT[:, ks, btok + off : btok + off + sz],
                                rhs=wqkv16[:, ks, 2 * C : 2 * C + 512],
                                start=(ks == 0),
                                stop=(ks == KS - 1),
                            )

                    def piece_b(ci=ci, off=off, sz=sz, holder=holder):
                        xT = xT_cur[g // 2]
                        tg = holder["tg"]
                        for ks in range(KS):
                            nc.tensor.matmul(
                                tg[:sz, 512:768],
                                lhsT=xT[:, ks, btok + off : btok + off + sz],
                                rhs=wqkv16[:, ks, 2 * C + 512 : 3 * C],
                                start=(ks == 0),
                                stop=(ks == KS - 1),
                            )
                        nc.scalar.copy(
                            va[:sz, sl, ci, :, 0:64],
                            tg[:sz, 0:768].rearrange("p (h d) -> p h d", d=64),
                        )

                    pieces.append(piece_a)
                    pieces.append(piece_b)
                return pieces

            def emit_D(g, fillers):
                """Scores + exp for batch g, interleaving filler pieces."""
                sl = g % 2
                psl = (g // 2) % 2
                btok = (g % 2) * N
                fi = 0
                nf = len(fillers)
                gi = 0
                for ci, (koff, ksz) in enumerate(KT_CHUNKS):
                    for hg in range(3):
                        # two heads per matmul: both heads' padded q side by
                        # side (N=512); the packed kT chunk's parity halves
                        # each hit their own head, zeros kill cross terms
                        psc = P["ps"].tile([128, 2, 2, 256], F32, tag="ps")
                        for hp in range(2):
                            h0 = hg * 4 + hp * 2
                            nc.tensor.matmul(
                                psc[:ksz, hp, :, :],
                                lhsT=kTpk[:, psl, h0 // 2, btok + koff : btok + koff + ksz],
                                rhs=qTp[:, psl, h0 : h0 + 2, btok + 64 : btok + 320],
                                start=True,
                                stop=True,
                            )
                        nc.scalar.activation(
                            es[:ksz, sl, ci, hg * 4 : hg * 4 + 4, :],
                            psc[:ksz, :, :, :].rearrange("p a b q -> p (a b) q"),
                            mybir.ActivationFunctionType.Exp,
                            scale=0.125,
                        )
                        gi += 1
                        want = (nf * gi) // 9
                        while fi < want:
                            fillers[fi]()
                            fi += 1
                while fi < len(fillers):
                    fillers[fi]()
                    fi += 1

            def make_E(g):
                """Template scores + exp for batch g (2 pieces)."""
                sl = g % 2
                psl = (g // 2) % 2
                btok = (g % 2) * N
                holder = {}

                def mm_piece():
                    tg = P["pg"].tile([128, 1024], F32, tag="pg")
                    holder["tg"] = tg
                    for hp in range(6):
                        h0 = 2 * hp
                        nc.tensor.matmul(
                            tg[0:64, h0 * 64 : (h0 + 2) * 64],
                            lhsT=kTpk[:, psl, hp, btok : btok + 64],
                            rhs=qTp[:, psl, h0 : h0 + 2, btok : btok + 64],
                            start=True,
                            stop=True,
                        )

                def exp_piece():
                    tg = holder["tg"]
                    nc.scalar.activation(
                        esm[0:64, sl, :, :],
                        tg[0:64, 0:768].rearrange("p (h q) -> p h q", q=64),
                        mybir.ActivationFunctionType.Exp,
                        scale=0.125,
                    )

                return [mm_piece, exp_piece]

            def _normalize(tg, qsz, qg, half, sl):
                po_v = tg[:qsz, 0:510].rearrange("p (h s) -> p h s", s=SLOT)
                rcp = P["rcp"].tile([128, 8], F32, tag="rcp")
                nc.vector.reciprocal(rcp[:qsz, 0:6], po_v[:, :, 64])
                nc.vector.tensor_tensor(
                    attn[:qsz, sl, qg, half * 384 : (half + 1) * 384].rearrange(
                        "p (h d) -> p h d", d=64
                    ),
                    po_v[:, :, 0:64],
                    rcp[:qsz, 0:6, None].to_broadcast([qsz, 6, 64]),
                    mybir.AluOpType.mult,
                )

            def make_FGH(g):
                """PV + normalize + attn^T + proj for batch g (deferred)."""
                sl = g % 2
                pieces = []

                # template PV (2 pieces, one per head-half)
                tpv_pieces = []
                for half in range(2):
                    def tpv(half=half):
                        tg = P["pg"].tile([128, 1024], F32, tag="pg")
                        for j in range(6):
                            h = half * 6 + j
                            nc.tensor.matmul(
                                tg[0:64, j * SLOT : j * SLOT + 65],
                                lhsT=esm[:, sl, h, 0:64],
                                rhs=va[:, sl, 0, h, 0:65],
                                start=True,
                                stop=True,
                            )
                        _normalize(tg, 64, 0, half, sl)
                    tpv_pieces.append(tpv)

                # search PV (4 pieces: qg x half)
                spv_pieces = {}
                for qg in (1, 2):
                    for half in range(2):
                        def spv(qg=qg, half=half):
                            tg = P["pg"].tile([128, 1024], F32, tag="pg")
                            for j in range(6):
                                h = half * 6 + j
                                for ci in range(3):
                                    nc.tensor.matmul(
                                        tg[0:128, j * SLOT : j * SLOT + 65],
                                        lhsT=es[:, sl, ci, h, (qg - 1) * 128 : qg * 128],
                                        rhs=va[:, sl, ci, h, 0:65],
                                        start=(ci == 0),
                                        stop=(ci == 2),
                                    )
                            _normalize(tg, 128, qg, half, sl)
                        spv_pieces[(qg, half)] = spv

                # attn^T via regular matmuls (6 pieces)
                at_pieces = []
                for fc in range(KS):
                    def at(fc=fc):
                        tg = P["pg"].tile([128, 1024], F32, tag="pg")
                        # overlap-packed: qg0 -> 0:128 (real 0:64), qg1 -> 64:192,
                        # qg2 -> 192:320
                        for qg, dst0 in ((0, 0), (1, 64), (2, 192)):
                            nc.tensor.matmul(
                                tg[:, dst0 : dst0 + 128],
                                lhsT=attn[0:128, sl, qg, fc * 128 : (fc + 1) * 128],
                                rhs=ident16[:, 0:128],
                                start=True,
                                stop=True,
                            )
                        nc.vector.tensor_copy(attnT[:, sl, fc, 0:N], tg[:, 0:N])
                    at_pieces.append(at)

                # proj + bias + out DMA (3 pieces)
                pieces = []
                row0 = g * N
                for qc, (qoff, qsz) in enumerate(P_CHUNKS):
                    def pj(qc=qc, qoff=qoff, qsz=qsz):
                        tg = P["pg"].tile([128, 1024], F32, tag="pg")
                        for ks in range(KS):
                            nc.tensor.matmul(
                                tg[:qsz, 0:512],
                                lhsT=attnT[:, sl, ks, qoff : qoff + qsz],
                                rhs=wproj16[:, ks, 0:512],
                                start=(ks == 0),
                                stop=(ks == KS - 1),
                            )
                        for ks in range(KS):
                            nc.tensor.matmul(
                                tg[:qsz, 512:768],
                                lhsT=attnT[:, sl, ks, qoff : qoff + qsz],
                                rhs=wproj16[:, ks, 512:768],
                                start=(ks == 0),
                                stop=(ks == KS - 1),
                            )
                        ost = P["outst"].tile([128, C], F16, tag="outst")
                        nc.vector.tensor_tensor(
                            ost[:qsz, :], tg[:qsz, 0:768], bias_bc[:qsz, :],
                            mybir.AluOpType.add,
                        )
                        nc.sync.dma_start(
                            out_ap[row0 + qoff : row0 + qoff + qsz, :], ost[:qsz, :]
                        )
                    pieces.append(pj)
                pj_pieces = pieces
                # order: half-0 PV -> attnT fc 0-2 -> half-1 PV -> attnT 3-5
                # -> proj; gets attn^T/proj flowing as early as possible
                return (
                    [tpv_pieces[0], spv_pieces[(1, 0)], spv_pieces[(2, 0)]]
                    + at_pieces[0:3]
                    + [tpv_pieces[1], spv_pieces[(1, 1)], spv_pieces[(2, 1)]]
                    + at_pieces[3:6]
                    + pj_pieces
                )

            # ================= main schedule =================
            # x DMAs first (unblock PE transposes ASAP), then the weight
            # block (HBM-bound), pads last so DVE drains xT promptly.
            xT_cur = {}
            xT_cur[0] = emit_xT(0)
            emit_weight_load()
            emit_pads()

            stash = []
            for p in range(NPAIR):
                for pc in stash:  # F/G/H of batch 2p-1
                    pc()
                stash = []
                if p + 1 < NPAIR:
                    xT_cur[p + 1] = emit_xT(p + 1)
                emit_B(p, xT_cur[p])
                g0, g1 = 2 * p, 2 * p + 1
                for pc in make_C(g0):
                    pc()
                fill0 = make_C(g1) + make_E(g0)
                emit_D(g0, fill0)
                stash0 = make_FGH(g0)
                emit_D(g1, stash0 + make_E(g1))
                stash = make_FGH(g1)
            for pc in stash:
                pc()

    nc.compile()
    return nc


@functools.cache
def _get_nc():
    return build_kernel()


def make_in_maps(x, wqkv, wproj, bias):
    x16 = x.reshape(B, N, C).astype(np.float16)
    wqkv16 = np.ascontiguousarray(wqkv.astype(np.float16))
    wproj16 = np.ascontiguousarray(wproj.astype(np.float16))
    bias = np.ascontiguousarray(bias.astype(np.float32))
    return [
        {
            "xT16": np.ascontiguousarray(
                x16[c * B_CORE : (c + 1) * B_CORE].reshape(TOK_CORE, C).T
            ),
            "W_qkv16": wqkv16,
            "W_proj16": wproj16,
            "b_proj": bias,
        }
        for c in range(NCORES)
    ]


def kernel(**inputs):
    x = np.ascontiguousarray(np.asarray(inputs["x"], dtype=np.float32))
    wqkv = np.ascontiguousarray(np.asarray(inputs["W_qkv"], dtype=np.float32))
    wproj = np.ascontiguousarray(np.asarray(inputs["W_proj"], dtype=np.float32))
    bias = np.ascontiguousarray(np.asarray(inputs["b_proj"], dtype=np.float32))
    t_h = int(inputs.get("t_h", 8))
    t_w = int(inputs.get("t_w", 8))
    assert t_h * t_w == 64, "kernel built for template length 64"
    assert x.shape == (B, N, C)

    nc = _get_nc()
    in_maps = make_in_maps(x, wqkv, wproj, bias)
    res = run_bass_kernel_spmd(nc, in_maps, core_ids=list(range(NCORES)))
    out = np.concatenate(
        [r["out"].astype(np.float32).reshape(B_CORE, N, C) for r in res.results],
        axis=0,
    )
    return out


if __name__ == "__main__":
    _get_nc()
    print("kernel_v2 built OK")
